# revision 1
# baseline (speedup 1.0000x reference)
"""GAT 2-layer encoder on 8 Trainium2 NeuronCores.

Reference computation: layer 1 = GAT conv over edge_index[:, :500] (weights W1),
layer 2 = GAT conv over edge_index[:, 500:] (weights W2).

Strategy:
  - Layer-1 output x1 differs from b1 only on the <=500 distinct dsts of the
    first 500 edges ("specials").  By linearity, layer 2's weighted aggregation
    commutes with the W2 transform, so layer 2 gathers x1-space rows and the
    gather table collapses to <=501 distinct 512B rows [x1 | asrc2 | adst2 | pad]
    (row 0 = default b1 row, rows 1..K = specials).  Indices then fit in int16
    for dma_gather.
  - Sharding: dst-range partition of the 1.6M layer-2 edges across 8 cores (no
    collectives; layer 1 + table build replicated on every core, it is tiny).
  - Per core: dsts sorted by in-degree, grouped into blocks of 128 (one dst per
    SBUF partition, its edges along the free dim, padded to the block max degree
    L).  One dma_gather per superblock fetches one 512B row per edge slot.
    Segment softmax = per-partition free-dim ops (DVE/ACT), weighted sum = DVE
    mul + strided reduce, final out = PE matmul [msgT;1] @ [W2;b2].
"""

import sys

sys.path.insert(0, "/opt/trn_rl_repo")

from contextlib import ExitStack

import numpy as np

import concourse.bacc as bacc
import concourse.bass as bass
import concourse.mybir as mybir
import concourse.tile as tile
from concourse.bass_utils import run_bass_kernel_spmd
from concourse.masks import make_identity

F32 = mybir.dt.float32
I16 = mybir.dt.int16
I32 = mybir.dt.int32
AF = mybir.ActivationFunctionType
OP = mybir.AluOpType

N = 100000
D = 64
NCORES = 8
NPC = N // NCORES          # dst nodes per core
P = 128
NSPLIT = 500               # first 500 edges -> layer 1
SMAX = 80                  # max edge-slots per superblock (SBUF budget)
NEG_SLOPE = 0.2
EPS = 1e-16
BIG = 200.0                # score shift so padded slots underflow exp to 0.0
GCHUNK = 32                # slots per packed gather call
PW = 4                     # slots packed per gather descriptor (PW*512B rows)


def _wrap16(flat):
    """int16 stream [n] (n%16==0) -> dma_gather idx tile [128, n//16]."""
    w = flat.reshape(-1, 16).T
    return np.ascontiguousarray(np.tile(w, (8, 1)).astype(np.int16))


def _grid(deg_sorted_max, npos):
    """Block structure from the (cross-core max) descending degree profile.

    Returns (L_b list, superblocks, groups):
      superblocks: dicts {b0, b1, S (slots), slot0}
      groups: dicts {sb, b0, B, L, slot_off (slots from sb start)}
    """
    nblocks = npos // P
    L = [max(int(deg_sorted_max[b * P]), 1) for b in range(nblocks)]
    sbs = []
    b = 0
    while b < nblocks:
        s = 0
        b0 = b
        while b < nblocks and (b - b0) < 16 and s + L[b] <= max(SMAX, L[b0]):
            s += L[b]
            b += 1
        sbs.append({"b0": b0, "b1": b, "S": s})
    slot0 = 0
    for sb in sbs:
        sb["slot0"] = slot0
        slot0 += sb["S"]
    groups = []
    for si, sb in enumerate(sbs):
        off = 0
        b = sb["b0"]
        while b < sb["b1"]:
            b0 = b
            while b < sb["b1"] and L[b] == L[b0]:
                b += 1
            groups.append({"sb": si, "b0": b0, "B": b - b0, "L": L[b0], "slot_off": off})
            off += (b - b0) * L[b0]
    return L, sbs, groups


VTAB = 1024               # gather table rows (specials + default replicas)


def _edge_streams(src, dst_local, rowmap_vals, npos, npc, Lb, sbs, repl_lo):
    """Per-partition edge grid for one core.

    Returns (eidx [128, 8*S_total] i16, mask [128, S_total] f32,
             degpos [128, nblocks] f32, order [npc])."""
    nblocks = npos // P
    deg = np.bincount(dst_local, minlength=npc)
    order = np.argsort(-deg, kind="stable")
    deg_sorted = deg[order]
    rank = np.empty(npc, np.int64)
    rank[order] = np.arange(npc)
    pos = rank[dst_local]
    pe = np.argsort(pos, kind="stable")
    pos_s = pos[pe]
    val_s = rowmap_vals[pe]
    start_of_pos = np.searchsorted(pos_s, np.arange(npos))
    k = np.arange(len(pos_s)) - start_of_pos[pos_s]
    blk = pos_s // P
    prt = pos_s % P
    slot_base = np.concatenate([[0], np.cumsum(Lb)])[:-1]
    s_global = slot_base[blk] + k
    S_total = int(sum(Lb))
    flat_j = s_global * P + prt
    rng = np.random.default_rng(12345)
    idxflat = rng.integers(repl_lo, VTAB, S_total * P).astype(np.int16)
    vs = val_s.astype(np.int16)
    zz = vs == 0
    vs[zz] = rng.integers(repl_lo, VTAB, int(zz.sum())).astype(np.int16)
    idxflat[flat_j] = vs
    maskflat = np.zeros(S_total * P, np.float32)
    maskflat[flat_j] = 1.0
    mask = np.ascontiguousarray(maskflat.reshape(S_total, P).T)
    eidx = np.concatenate(
        [_wrap16(idxflat[sb["slot0"] * P:(sb["slot0"] + sb["S"]) * P]) for sb in sbs],
        axis=1,
    )
    degpad = np.zeros(npos, np.float32)
    degpad[:npc] = deg_sorted
    degpos = np.ascontiguousarray((degpad > 0).astype(np.float32).reshape(nblocks, P).T)
    return eidx, mask, degpos, order, idxflat


def prep(inputs):
    """Host-side index prep (pure index computation, no feature values)."""
    ei = np.asarray(inputs["edge_index"])
    src = ei[0].astype(np.int64)
    dst = ei[1].astype(np.int64)
    s1, d1 = src[:NSPLIT], dst[:NSPLIT]
    s2, d2 = src[NSPLIT:], dst[NSPLIT:]

    # ---- layer 1 structure ----
    specials, deg1 = np.unique(d1, return_counts=True)
    K = len(specials)
    order1 = np.argsort(-deg1, kind="stable")
    spec_by_pos = specials[order1]          # grid position q -> node, table row q+1
    rowmap = np.zeros(N, np.int16)
    rowmap[spec_by_pos] = np.arange(1, K + 1)
    K1pos = K + 1                            # one guaranteed pad slot (default row)
    nblk1 = (K1pos + P - 1) // P
    npos1 = nblk1 * P

    U = np.unique(np.concatenate([s1, d1]))
    nU = len(U)
    nUt = (nU + P - 1) // P
    uidx = np.zeros((P, nUt), np.int32)
    upad = np.zeros(nUt * P, np.int64)
    upad[:nU] = U
    uidx[:, :] = upad.reshape(nUt, P).T
    uindex = np.zeros(N, np.int64)
    uindex[U] = np.arange(nU)

    # layer-1 edge grid (dst -> grid position via rank over specials)
    rank1 = np.empty(K, np.int64)
    rank1[order1] = np.arange(K)
    d1pos = rank1[np.searchsorted(specials, d1)]
    deg1_sorted = np.zeros(npos1, np.int64)
    deg1_sorted[:K] = deg1[order1]
    L1, sbs1, groups1 = _grid(deg1_sorted, npos1)
    S1 = int(sum(L1))
    # edge stream for layer 1 (single "core")
    pe = np.argsort(d1pos, kind="stable")
    pos_s = d1pos[pe]
    val_s = uindex[s1[pe]].astype(np.int16)
    start_of_pos = np.searchsorted(pos_s, np.arange(npos1))
    k = np.arange(len(pos_s)) - start_of_pos[pos_s]
    slot_base = np.concatenate([[0], np.cumsum(L1)])[:-1]
    flat_j = (slot_base[pos_s // P] + k) * P + (pos_s % P)
    idxflat = np.zeros(S1 * P, np.int16)
    idxflat[flat_j] = val_s
    maskflat = np.zeros(S1 * P, np.float32)
    maskflat[flat_j] = 1.0
    l1_mask = np.ascontiguousarray(maskflat.reshape(S1, P).T)
    l1_eidx = np.concatenate(
        [_wrap16(idxflat[sb["slot0"] * P:(sb["slot0"] + sb["S"]) * P]) for sb in sbs1],
        axis=1,
    )
    dv1 = np.zeros(npos1, np.int16)
    dv1[:K] = uindex[spec_by_pos]
    l1_didx = np.concatenate(
        [_wrap16(dv1[P * sb["b0"]:P * sb["b1"]]) for sb in sbs1], axis=1
    )
    dp1 = np.zeros(npos1, np.float32)
    dp1[:K] = (deg1[order1] > 0)
    l1_degpos = np.ascontiguousarray(dp1.reshape(nblk1, P).T)

    # ---- layer 2 structure ----
    npos = ((NPC + P - 1) // P) * P
    core_dat = []
    deg_sorted_all = np.zeros(npos, np.int64)
    for c in range(NCORES):
        sel = (d2 >= c * NPC) & (d2 < (c + 1) * NPC)
        dl = d2[sel] - c * NPC
        sl = s2[sel]
        deg = np.bincount(dl, minlength=NPC)
        ds = np.sort(deg)[::-1]
        m = min(NPC, npos)
        deg_sorted_all[:m] = np.maximum(deg_sorted_all[:m], ds[:m])
        core_dat.append((sl, dl))
    L2, sbs2, groups2 = _grid(deg_sorted_all, npos)
    dcol = 0
    for sb in sbs2:
        nblk_sb = sb["b1"] - sb["b0"]
        sb["nb4"] = ((nblk_sb + PW - 1) // PW) * PW
        sb["dcol0"] = dcol
        dcol += sb["nb4"] // PW
    dtot = dcol
    # force slot-count per superblock to a multiple of PW so rows pack cleanly
    for sb in sbs2:
        r = (-sb["S"]) % PW
        if r:
            L2[sb["b1"] - 1] += r
            sb["S"] += r
    slot0 = 0
    for sb in sbs2:
        sb["slot0"] = slot0
        slot0 += sb["S"]
    groups2 = []
    for si, sb in enumerate(sbs2):
        off = 0
        b = sb["b0"]
        while b < sb["b1"]:
            b0 = b
            while b < sb["b1"] and L2[b] == L2[b0]:
                b += 1
            groups2.append({"sb": si, "b0": b0, "B": b - b0, "L": L2[b0],
                            "slot_off": off})
            off += (b - b0) * L2[b0]
    S2 = int(sum(L2))
    nblk2 = npos // P

    cores = []
    lo_pack = (K + PW) // PW    # first all-default packed row in the packed view
    for c in range(NCORES):
        sl, dl = core_dat[c]
        eidx, mask, degpos, order, idxflat = _edge_streams(
            sl, dl, rowmap[sl], npos, NPC, L2, sbs2, K + 1
        )
        # pack PW consecutive slots per partition; all-default packs read one
        # PW*512B replica row, mixed packs read an on-device-built pairfix row
        rngp = np.random.default_rng(4242 + c)
        pidx_segs = []
        pfix_vals = []
        for sb in sbs2:
            s0, S = sb["slot0"], sb["S"]
            iv = idxflat[s0 * P:(s0 + S) * P].reshape(S // PW, PW, P)
            pp = rngp.integers(lo_pack, VTAB // PW,
                               (S // PW, P)).astype(np.int16)
            mixed = (iv <= K).any(axis=1)
            nm = int(mixed.sum())
            if nm:
                pp[mixed] = (VTAB // PW + len(pfix_vals) // PW
                             + np.arange(nm)).astype(np.int16)
                mv = np.moveaxis(iv, 1, 2)[mixed].reshape(-1)
                pfix_vals.extend(mv.tolist())
            pidx_segs.append(_wrap16(pp.reshape(-1)))
        pidx = np.concatenate(pidx_segs, axis=1)
        rngd = np.random.default_rng(777 + c)
        dv = rngd.integers(K + 1, VTAB, npos).astype(np.int16)
        dvr = rowmap[c * NPC + order]
        dz = dvr == 0
        dvr = dvr.copy()
        dvr[dz] = rngd.integers(K + 1, VTAB, int(dz.sum())).astype(np.int16)
        dv[:NPC] = dvr
        dsegs = []
        for sb in sbs2:
            nblk_sb = sb["b1"] - sb["b0"]
            nb4 = sb["nb4"]
            vals = np.full((nb4, P), 0, np.int16)
            vals[:nblk_sb] = dv[P * sb["b0"]:P * sb["b1"]].reshape(nblk_sb, P)
            if nb4 > nblk_sb:
                vals[nblk_sb:] = rngd.integers(
                    K + 1, VTAB, (nb4 - nblk_sb, P)).astype(np.int16)
            v4 = vals.reshape(nb4 // PW, PW, P)
            pp = rngd.integers(lo_pack, VTAB // PW,
                               (nb4 // PW, P)).astype(np.int16)
            mixed = (v4 <= K).any(axis=1)
            nm = int(mixed.sum())
            if nm:
                pp[mixed] = (VTAB // PW + len(pfix_vals) // PW
                             + np.arange(nm)).astype(np.int16)
                pfix_vals.extend(np.moveaxis(v4, 1, 2)[mixed].reshape(-1).tolist())
            dsegs.append(_wrap16(pp.reshape(-1)))
        didx = np.concatenate(dsegs, axis=1)
        cores.append({"eidx": eidx, "mask": mask, "degpos": degpos,
                      "didx": didx, "order": order, "pidx": pidx,
                      "pfix": np.asarray(pfix_vals, np.int16)})
    # common pairfix region size across cores (SPMD program is shared)
    npf = max((len(c["pfix"]) for c in cores), default=0)
    Spf = max((npf + P - 1) // P, 1)
    for c in cores:
        pf = np.zeros(Spf * P, np.int16)
        pf[:len(c["pfix"])] = c["pfix"]
        c["pfidx"] = _wrap16(pf)

    meta = {
        "K": K, "K1pos": K1pos, "nblk1": nblk1, "nU": nU, "nUt": nUt,
        "L1": L1, "sbs1": sbs1, "groups1": groups1, "S1": S1,
        "L2": L2, "sbs2": sbs2, "groups2": groups2, "S2": S2, "nblk2": nblk2,
        "npos": npos, "Spf": Spf, "dtot": dtot,
    }
    l1 = {"uidx": uidx, "l1_eidx": l1_eidx, "l1_didx": l1_didx,
          "l1_mask": l1_mask, "l1_degpos": l1_degpos}
    return meta, l1, cores


def _emit_group(nc, gw, Gap, mask_ap, adst_ap, degpos_ap, B, L):
    """Segment softmax + weighted sum for B blocks of equal padded degree L.

    Gap: AP view [128, B*L, 128] of the gathered rows (slot-flat).
    Returns msg tile [128, B, 64]."""
    BL = B * L
    asrc = Gap[:, :, 64:65].rearrange("p s o -> p (s o)")        # [128, BL]
    s_t = gw.tile([P, B, L], F32, tag="s_t")
    nc.vector.tensor_tensor(s_t[:], asrc, adst_ap.to_broadcast((P, B, L)),
                            op=OP.add)
    u_t = gw.tile([P, B, L], F32, tag="u_t")
    nc.vector.scalar_tensor_tensor(u_t[:], s_t[:], NEG_SLOPE, s_t[:],
                                   op0=OP.mult, op1=OP.max)
    e2_t = gw.tile([P, B, L], F32, tag="e2_t")
    nc.vector.scalar_tensor_tensor(e2_t[:], u_t[:], BIG, mask_ap,
                                   op0=OP.add, op1=OP.mult)
    mneg = gw.tile([P, B], F32, tag="mneg")
    nc.vector.tensor_reduce(mneg[:], e2_t[:], axis=mybir.AxisListType.X,
                            op=OP.max, negate=True)
    d_t = gw.tile([P, B, L], F32, tag="d_t")
    nc.vector.tensor_tensor(d_t[:], e2_t[:], mneg[:].to_broadcast((P, B, L)),
                            op=OP.add)
    ex_t = gw.tile([P, B, L], F32, tag="ex_t")
    nc.scalar.activation(ex_t[:], d_t[:], AF.Exp)
    ssum = gw.tile([P, B], F32, tag="ssum")
    nc.vector.tensor_reduce(ssum[:], ex_t[:], axis=mybir.AxisListType.X,
                            op=OP.add)
    sp = gw.tile([P, B], F32, tag="sp")
    nc.vector.tensor_scalar_add(sp[:], ssum[:], EPS)
    rs = gw.tile([P, B], F32, tag="rs")
    nc.vector.reciprocal(rs[:], sp[:])
    rsd = gw.tile([P, B], F32, tag="rsd")
    nc.vector.tensor_tensor(rsd[:], rs[:], degpos_ap, op=OP.mult)
    alpha = gw.tile([P, B, L], F32, tag="alpha")
    nc.vector.tensor_tensor(alpha[:], ex_t[:], rsd[:].to_broadcast((P, B, L)),
                            op=OP.mult)
    wr = gw.tile([P, BL, D], F32, tag="wr")
    nc.vector.tensor_tensor(wr[:], Gap[:, :, 0:D],
                            alpha[:].rearrange("p b l -> p (b l)")
                            .to_broadcast((P, BL, D)), op=OP.mult)
    msg = gw.tile([P, B, D], F32, tag="msg")
    nc.vector.tensor_reduce(msg[:], wr[:].rearrange("p (b l) f -> p b f l", b=B),
                            axis=mybir.AxisListType.X, op=OP.add)
    return msg


def build(meta, repeat=1, limit_sb=None, debug_lvl=3, gchunk=GCHUNK):
    """Build the SPMD Bass program (common across cores)."""
    K = meta["K"]
    nblk1, nUt = meta["nblk1"], meta["nUt"]
    S1, sbs1, groups1, L1 = meta["S1"], meta["sbs1"], meta["groups1"], meta["L1"]
    S2, sbs2, groups2, L2 = meta["S2"], meta["sbs2"], meta["groups2"], meta["L2"]
    nblk2 = meta["nblk2"]

    nc = bacc.Bacc("TRN2", target_bir_lowering=False, debug=False,
                   num_devices=NCORES)
    dt = nc.dram_tensor
    x_in = dt("x_in", [N, D], F32, kind="ExternalInput").ap()
    W1_in = dt("W1_in", [D, D], F32, kind="ExternalInput").ap()
    W1T_in = dt("W1T_in", [D, D], F32, kind="ExternalInput").ap()
    W2_in = dt("W2_in", [D, D], F32, kind="ExternalInput").ap()
    W2T_in = dt("W2T_in", [D, D], F32, kind="ExternalInput").ap()
    av1_in = dt("av1_in", [D, 2], F32, kind="ExternalInput").ap()
    av2_in = dt("av2_in", [D, 2], F32, kind="ExternalInput").ap()
    b1row_in = dt("b1row_in", [1, D], F32, kind="ExternalInput").ap()
    b2row_in = dt("b2row_in", [1, D], F32, kind="ExternalInput").ap()
    b1col_in = dt("b1col_in", [D, 1], F32, kind="ExternalInput").ap()
    uidx_in = dt("uidx_in", [P, nUt], I32, kind="ExternalInput").ap()
    l1_eidx_in = dt("l1_eidx_in", [P, 8 * S1], I16, kind="ExternalInput").ap()
    l1_didx_in = dt("l1_didx_in", [P, 8 * nblk1], I16, kind="ExternalInput").ap()
    l1_mask_in = dt("l1_mask_in", [P, S1], F32, kind="ExternalInput").ap()
    l1_degpos_in = dt("l1_degpos_in", [P, nblk1], F32, kind="ExternalInput").ap()
    Spf = meta["Spf"]
    pidx_in = dt("pidx_in", [P, 8 * (S2 // PW)], I16, kind="ExternalInput").ap()
    pfidx_in = dt("pfidx_in", [P, 8 * Spf], I16, kind="ExternalInput").ap()
    didx_in = dt("didx_in", [P, 8 * meta["dtot"]], I16, kind="ExternalInput").ap()
    mask_in = dt("mask_in", [P, S2], F32, kind="ExternalInput").ap()
    degpos_in = dt("degpos_in", [P, nblk2], F32, kind="ExternalInput").ap()
    out_t = dt("out", [meta["npos"], D], F32, kind="ExternalOutput").ap()

    h1tab = dt("h1tab", [nUt * P, P], F32).ap()
    tab = dt("tab", [VTAB + Spf * P, P], F32).ap()

    with tile.TileContext(nc) as tc, ExitStack() as ctx:
        const = ctx.enter_context(tc.tile_pool(name="const", bufs=1))
        psc_ctx = tc.tile_pool(name="psc", bufs=1, space="PSUM")
        psc = psc_ctx.__enter__()

        ident = const.tile([P, P], F32)
        make_identity(nc, ident[:])

        # ---- weights / augmented matrices ----
        W1s = const.tile([D, D], F32)
        nc.sync.dma_start(W1s[:], W1_in[:])
        W1Ts = const.tile([D, D], F32)
        nc.sync.dma_start(W1Ts[:], W1T_in[:])
        W2s = const.tile([D, D], F32)
        nc.sync.dma_start(W2s[:], W2_in[:])
        W2Ts = const.tile([D, D], F32)
        nc.sync.dma_start(W2Ts[:], W2T_in[:])
        av1s = const.tile([D, 2], F32)
        nc.sync.dma_start(av1s[:], av1_in[:])
        av2s = const.tile([D, 2], F32)
        nc.sync.dma_start(av2s[:], av2_in[:])
        b1cols = const.tile([D, 1], F32)
        nc.sync.dma_start(b1cols[:], b1col_in[:])

        wt1_p = psc.tile([D, 2], F32, space="PSUM")
        nc.tensor.matmul(wt1_p[:], W1Ts[:], av1s[:], start=True, stop=True)
        wt2_p = psc.tile([D, 2], F32, space="PSUM")
        nc.tensor.matmul(wt2_p[:], W2Ts[:], av2s[:], start=True, stop=True)
        wt2s = const.tile([D, 2], F32)
        nc.vector.tensor_copy(wt2s[:], wt2_p[:])

        W1aug = const.tile([D, D + 2], F32)
        nc.vector.tensor_copy(W1aug[:, 0:D], W1s[:])
        nc.vector.tensor_copy(W1aug[:, D:D + 2], wt1_p[:])

        # SPEC [65, 66] = [[I | wt2s | wt2d]; [b1 | b1.wt2s | b1.wt2d]]
        SPEC = const.tile([D + 1, D + 2], F32)
        nc.vector.tensor_copy(SPEC[0:D, 0:D], ident[0:D, 0:D])
        nc.vector.tensor_copy(SPEC[0:D, D:D + 2], wt2s[:])
        nc.sync.dma_start(SPEC[D:D + 1, 0:D], b1row_in[:])
        b1w_p = psc.tile([1, 2], F32, space="PSUM")
        nc.tensor.matmul(b1w_p[:], b1cols[:], wt2s[:], start=True, stop=True)
        nc.vector.tensor_copy(SPEC[D:D + 1, D:D + 2], b1w_p[:])

        W2OUT = const.tile([D + 1, D], F32)
        nc.vector.tensor_copy(W2OUT[0:D, :], W2s[:])
        nc.sync.dma_start(W2OUT[D:D + 1, :], b2row_in[:])

        psc_ctx.__exit__(None, None, None)

        # ---- layer 1: build h1 table for the U endpoint nodes ----
        uidx_s = const.tile([P, nUt], I32)
        nc.sync.dma_start(uidx_s[:], uidx_in[:])
        with tc.tile_pool(name="l1u", bufs=2) as l1u, \
             tc.tile_pool(name="l1up", bufs=2, space="PSUM") as l1up:
            for t in range(nUt):
                xU = l1u.tile([P, D], F32, tag="xU")
                nc.gpsimd.indirect_dma_start(
                    out=xU[:], out_offset=None, in_=x_in[:, :],
                    in_offset=bass.IndirectOffsetOnAxis(ap=uidx_s[:, t:t + 1], axis=0))
                xT_p = l1up.tile([D, P], F32, space="PSUM", tag="xT")
                nc.tensor.transpose(xT_p[:], xU[:], ident[:])
                xT_s = l1u.tile([D, P], F32, tag="xTs")
                nc.vector.tensor_copy(xT_s[:], xT_p[:])
                h_p = l1up.tile([P, D + 2], F32, space="PSUM", tag="h_p")
                nc.tensor.matmul(h_p[:], xT_s[:], W1aug[:], start=True, stop=True)
                h_s = l1u.tile([P, P], F32, tag="h_s")
                nc.scalar.copy(h_s[:, 0:D + 2], h_p[:])
                nc.vector.memset(h_s[:, D + 2:P], 0.0)
                nc.sync.dma_start(h1tab[t * P:(t + 1) * P, :], h_s[:])

        # ---- layer 1 conv -> write table rows ----
        l1_eidx_s = const.tile([P, 8 * S1], I16)
        nc.sync.dma_start(l1_eidx_s[:], l1_eidx_in[:])
        l1_didx_s = const.tile([P, 8 * nblk1], I16)
        nc.sync.dma_start(l1_didx_s[:], l1_didx_in[:])
        l1_mask_s = const.tile([P, S1], F32)
        nc.sync.dma_start(l1_mask_s[:], l1_mask_in[:])
        l1_degpos_s = const.tile([P, nblk1], F32)
        nc.sync.dma_start(l1_degpos_s[:], l1_degpos_in[:])

        with tc.tile_pool(name="l1w", bufs=2) as l1w, \
             tc.tile_pool(name="l1p", bufs=2, space="PSUM") as l1p:
            dr1 = l1w.tile([P, nblk1, P], F32, tag="dr1")
            nc.gpsimd.dma_gather(dr1[:], h1tab[:, :], l1_didx_s[:],
                                 nblk1 * P, nblk1 * P, P, single_packet=False)
            adst1 = l1w.tile([P, nblk1], F32, tag="adst1")
            nc.scalar.activation(adst1[:],
                                 dr1[:, 0:nblk1, 65:66].rearrange("p b o -> p (b o)"),
                                 AF.Identity)
            for sb_i, sb in enumerate(sbs1):
                G1 = l1w.tile([P, sb["S"], P], F32, tag="G1")
                nc.gpsimd.dma_gather(
                    G1[:], h1tab[:, :],
                    l1_eidx_s[:, 8 * sb["slot0"]:8 * (sb["slot0"] + sb["S"])],
                    sb["S"] * P, sb["S"] * P, P, single_packet=False)
                for g in [g for g in groups1 if g["sb"] == sb_i]:
                    B, L, off = g["B"], g["L"], g["slot_off"]
                    sl0 = sb["slot0"] + off
                    msg = _emit_group(
                        nc, l1w, G1[:, off:off + B * L, :],
                        l1_mask_s[:, sl0:sl0 + B * L],
                        adst1[:, g["b0"]:g["b0"] + B],
                        l1_degpos_s[:, g["b0"]:g["b0"] + B], B, L)
                    for j in range(B):
                        b = g["b0"] + j
                        mT_p = l1p.tile([D, P], F32, space="PSUM", tag="mT")
                        nc.tensor.transpose(mT_p[:], msg[:, j, :], ident[:])
                        mT_s = l1w.tile([D + 1, P], F32, tag="mTs")
                        nc.vector.tensor_copy(mT_s[0:D, :], mT_p[:])
                        nc.vector.memset(mT_s[D:D + 1, :], 1.0)
                        row_p = l1p.tile([P, D + 2], F32, space="PSUM", tag="rowp")
                        nc.tensor.matmul(row_p[:], mT_s[:], SPEC[:],
                                         start=True, stop=True)
                        row_s = l1w.tile([P, P], F32, tag="rows")
                        nc.scalar.copy(row_s[:, 0:D + 2], row_p[:])
                        nc.vector.memset(row_s[:, D + 2:P], 0.0)
                        nrows = min(P, K - b * P)
                        if nrows > 0:
                            nc.sync.dma_start(
                                tab[1 + b * P:1 + b * P + nrows, :],
                                row_s[0:nrows, :])
                        if b == K // P:   # default row from the pad position K
                            q = K % P
                            nc.sync.dma_start(tab[0:1, :], row_s[q:q + 1, :])
                            # replicate the default row over rows K+1..VTAB-1
                            # (spreads the 99%-default gather traffic across
                            # HBM addresses instead of hammering one row)
                            zidx = l1w.tile([P, 8], I16, tag="zidx")
                            nc.vector.memset(zidx[:], 0)
                            defbc = l1w.tile([P, 1, P], F32, tag="defbc")
                            nc.gpsimd.dma_gather(defbc[:], tab[:, :], zidx[:],
                                                 P, P, P, single_packet=False)
                            r0 = K + 1
                            while r0 < VTAB:
                                cnt = min(P, VTAB - r0)
                                nc.sync.dma_start(tab[r0:r0 + cnt, :],
                                                  defbc[0:cnt, 0, :])
                                r0 += cnt

        # ---- build pairfix rows: [row(a) | row(b)] for mixed pairs ----
        pfidx_s = const.tile([P, 8 * Spf], I16)
        nc.sync.dma_start(pfidx_s[:], pfidx_in[:])
        with tc.tile_pool(name="pfw", bufs=1) as pfw:
            pfg = pfw.tile([P, Spf, P], F32)
            nc.gpsimd.dma_gather(pfg[:], tab[0:VTAB, :], pfidx_s[:],
                                 Spf * P, Spf * P, P, single_packet=False)
            nc.sync.dma_start(
                tab[VTAB:VTAB + Spf * P, :].rearrange("(s p) f -> p s f", p=P),
                pfg[:])

        # ---- layer 2 ----
        tp = tab[:].rearrange("(r w) f -> r (w f)", w=PW)
        pidx_s = const.tile([P, 8 * (S2 // PW)], I16)
        nc.sync.dma_start(pidx_s[:], pidx_in[:])
        didx_s = const.tile([P, 8 * meta["dtot"]], I16)
        nc.sync.dma_start(didx_s[:], didx_in[:])
        mask_s = const.tile([P, S2], F32)
        nc.sync.dma_start(mask_s[:], mask_in[:])
        degpos_s = const.tile([P, nblk2], F32)
        nc.sync.dma_start(degpos_s[:], degpos_in[:])

        with tc.tile_pool(name="sbw", bufs=2) as sbw, \
             tc.tile_pool(name="gw", bufs=2) as gw, \
             tc.tile_pool(name="blk", bufs=3) as blk, \
             tc.tile_pool(name="psb", bufs=3, space="PSUM") as psb:
            sbs2_run = sbs2 if limit_sb is None else sbs2[:limit_sb]
            for _rep in range(repeat):
                for sb_i, sb in enumerate(sbs2_run):
                    nblk_sb = sb["b1"] - sb["b0"]
                    hS = sb["S"] // PW
                    pidx_t = pidx_s[:, 8 * (sb["slot0"] // PW):
                                    8 * (sb["slot0"] // PW + hS)]
                    G = sbw.tile([P, sb["S"], P], F32, tag="G")
                    for off in range(0, sb["S"], gchunk):
                        cs = min(gchunk, sb["S"] - off)
                        Gv = G[:, off:off + cs, :].rearrange(
                            "p (k w) f -> p k (w f)", w=PW)
                        nc.gpsimd.dma_gather(
                            Gv, tp, pidx_t[:, 8 * (off // PW):
                                           8 * ((off + cs) // PW)],
                            cs // PW * P, cs // PW * P, PW * P,
                            single_packet=False)
                    nb4 = sb["nb4"]
                    dr = sbw.tile([P, nb4, P], F32, tag="dr")
                    nc.gpsimd.dma_gather(
                        dr[:].rearrange("p (k w) f -> p k (w f)", w=PW), tp,
                        didx_s[:, 8 * sb["dcol0"]:8 * (sb["dcol0"] + nb4 // PW)],
                        nb4 // PW * P, nb4 // PW * P, PW * P,
                        single_packet=False)
                    adst = sbw.tile([P, nblk_sb], F32, tag="adst")
                    nc.scalar.activation(
                        adst[:],
                        dr[:, 0:nblk_sb, 65:66].rearrange("p b o -> p (b o)"),
                        AF.Identity)
                    if debug_lvl < 2:
                        dum = sbw.tile([P, P], F32, tag="dum")
                        nc.vector.tensor_copy(dum[:], G[:, 0, :])
                        continue
                    for g in [g for g in groups2 if g["sb"] == sb_i]:
                        B, L, off = g["B"], g["L"], g["slot_off"]
                        sl0 = sb["slot0"] + off
                        msg = _emit_group(
                            nc, gw, G[:, off:off + B * L, :],
                            mask_s[:, sl0:sl0 + B * L],
                            adst[:, g["b0"] - sb["b0"]:g["b0"] - sb["b0"] + B],
                            degpos_s[:, g["b0"]:g["b0"] + B], B, L)
                        if debug_lvl < 3:
                            dum2 = blk.tile([P, D], F32, tag="dum2")
                            nc.vector.tensor_copy(dum2[:], msg[:, 0, :])
                            continue
                        for j in range(B):
                            b = g["b0"] + j
                            mT_p = psb.tile([D, P], F32, space="PSUM", tag="mT")
                            nc.tensor.transpose(mT_p[:], msg[:, j, :], ident[:])
                            mT_s = blk.tile([D + 1, P], F32, tag="mTs")
                            nc.vector.tensor_copy(mT_s[0:D, :], mT_p[:])
                            nc.vector.memset(mT_s[D:D + 1, :], 1.0)
                            o_p = psb.tile([P, D], F32, space="PSUM", tag="op")
                            nc.tensor.matmul(o_p[:], mT_s[:], W2OUT[:],
                                             start=True, stop=True)
                            o_s = blk.tile([P, D], F32, tag="os")
                            nc.scalar.copy(o_s[:], o_p[:])
                            nc.sync.dma_start(out_t[b * P:(b + 1) * P, :], o_s[:])

    nc.compile()
    return nc


def make_in_maps(inputs, meta, l1, cores):
    x = np.ascontiguousarray(np.asarray(inputs["x"], dtype=np.float32))
    W1 = np.asarray(inputs["W1"], dtype=np.float32)
    W2 = np.asarray(inputs["W2"], dtype=np.float32)
    base = {
        "x_in": x,
        "W1_in": np.ascontiguousarray(W1),
        "W1T_in": np.ascontiguousarray(W1.T),
        "W2_in": np.ascontiguousarray(W2),
        "W2T_in": np.ascontiguousarray(W2.T),
        "av1_in": np.ascontiguousarray(np.stack(
            [np.asarray(inputs["a_src1"]), np.asarray(inputs["a_dst1"])],
            axis=1).astype(np.float32)),
        "av2_in": np.ascontiguousarray(np.stack(
            [np.asarray(inputs["a_src2"]), np.asarray(inputs["a_dst2"])],
            axis=1).astype(np.float32)),
        "b1row_in": np.asarray(inputs["b1"], dtype=np.float32).reshape(1, D),
        "b2row_in": np.asarray(inputs["b2"], dtype=np.float32).reshape(1, D),
        "b1col_in": np.asarray(inputs["b1"], dtype=np.float32).reshape(D, 1),
        "uidx_in": l1["uidx"],
        "l1_eidx_in": l1["l1_eidx"],
        "l1_didx_in": l1["l1_didx"],
        "l1_mask_in": l1["l1_mask"],
        "l1_degpos_in": l1["l1_degpos"],
    }
    in_maps = []
    for c in range(NCORES):
        m = dict(base)
        m["pidx_in"] = cores[c]["pidx"]
        m["pfidx_in"] = cores[c]["pfidx"]
        m["didx_in"] = cores[c]["didx"]
        m["mask_in"] = cores[c]["mask"]
        m["degpos_in"] = cores[c]["degpos"]
        in_maps.append(m)
    return in_maps


def unshard(results, cores):
    out = np.empty((N, D), np.float32)
    for c in range(NCORES):
        oc = results[c]["out"]
        order = cores[c]["order"]
        out[c * NPC + order] = oc[:NPC]
    return out


def kernel(**inputs):
    meta, l1, cores = prep(inputs)
    nc = build(meta, repeat=1)
    in_maps = make_in_maps(inputs, meta, l1, cores)
    res = run_bass_kernel_spmd(nc, in_maps, core_ids=list(range(NCORES)))
    return unshard(res.results, cores)



# revision 2
# speedup vs baseline: 5.0137x; 5.0137x over previous
"""GAT 2-layer encoder on 8 Trainium2 NeuronCores.

Reference computation: layer 1 = GAT conv over edge_index[:, :500] (weights W1),
layer 2 = GAT conv over edge_index[:, 500:] (weights W2).

Strategy (sparse-special):
  - Layer-1 output x1 differs from b1 only on the K<=500 distinct dsts of the
    first 500 edges ("specials").  In layer 2, h2[src] = x1[src]@W2 is the
    constant default row for every non-special src, so only edges whose src is
    special (~8k of 1.6M) carry information.  For a dst with no special
    in-edge, softmax over equal scores gives alpha = 1/deg for every in-edge,
    hence out = b1@W2 + b2 exactly (up to the 1e-16 eps), a CONSTANT row.
  - Device builds a (K+2)-row table in h2-space with b2 baked in:
    row r = [x1_r@W2 + b2 | asrc2_r | adst2_r], row 0 = default, row K+1 = b2
    (for deg-0 dsts), rows K+2..VTAB = replicas of row 0 (spread gather load).
  - Sharding: dst-range partition of the 1.6M layer-2 edges across 8 cores (no
    collectives; layer 1 + table build replicated on every core, it is tiny).
  - Per core: dsts sorted by special-in-degree; only the first ~9 blocks of 128
    ("computed region") run the segment softmax over a tiny slot grid; each dst
    gets [special slots | default slot (weight = #default in-edges) | dst slot
    (carries adst2)].  The remaining ~89 blocks are written with the constant
    default row.  No matmul needed in layer 2: table rows are pre-transformed.
"""

import sys

sys.path.insert(0, "/opt/trn_rl_repo")

from contextlib import ExitStack

import numpy as np

import concourse.bacc as bacc
import concourse.bass as bass
import concourse.mybir as mybir
import concourse.tile as tile
from concourse.bass_utils import run_bass_kernel_spmd
from concourse.masks import make_identity

F32 = mybir.dt.float32
I16 = mybir.dt.int16
I32 = mybir.dt.int32
AF = mybir.ActivationFunctionType
OP = mybir.AluOpType

N = 100000
D = 64
NCORES = 8
NPC = N // NCORES          # dst nodes per core
P = 128
NSPLIT = 500               # first 500 edges -> layer 1
NEG_SLOPE = 0.2
EPS = 1e-16
BIG = 200.0                # score shift so padded slots underflow exp to 0.0
VTAB = 1024                # gather table rows (specials + default replicas)
DB = 8                     # blocks per default-row broadcast write


def _wrap16(flat):
    """int16 stream [n] (n%16==0) -> dma_gather idx tile [128, n//16]."""
    w = flat.reshape(-1, 16).T
    return np.ascontiguousarray(np.tile(w, (8, 1)).astype(np.int16))


def _grid(deg_sorted_max, npos, smax=80):
    """Block structure from the descending degree profile (layer 1)."""
    nblocks = npos // P
    L = [max(int(deg_sorted_max[b * P]), 1) for b in range(nblocks)]
    sbs = []
    b = 0
    while b < nblocks:
        s = 0
        b0 = b
        while b < nblocks and (b - b0) < 16 and s + L[b] <= max(smax, L[b0]):
            s += L[b]
            b += 1
        sbs.append({"b0": b0, "b1": b, "S": s})
    slot0 = 0
    for sb in sbs:
        sb["slot0"] = slot0
        slot0 += sb["S"]
    groups = []
    for si, sb in enumerate(sbs):
        off = 0
        b = sb["b0"]
        while b < sb["b1"]:
            b0 = b
            while b < sb["b1"] and L[b] == L[b0]:
                b += 1
            groups.append({"sb": si, "b0": b0, "B": b - b0, "L": L[b0], "slot_off": off})
            off += (b - b0) * L[b0]
    return L, sbs, groups


def prep(inputs):
    """Host-side index prep (pure index computation, no feature values)."""
    ei = np.asarray(inputs["edge_index"])
    src = ei[0].astype(np.int64)
    dst = ei[1].astype(np.int64)
    s1, d1 = src[:NSPLIT], dst[:NSPLIT]
    s2, d2 = src[NSPLIT:], dst[NSPLIT:]

    # ---- layer 1 structure ----
    specials, deg1 = np.unique(d1, return_counts=True)
    K = len(specials)
    order1 = np.argsort(-deg1, kind="stable")
    spec_by_pos = specials[order1]          # grid position q -> node, table row q+1
    rowmap = np.zeros(N, np.int16)
    rowmap[spec_by_pos] = np.arange(1, K + 1)
    K1pos = K + 1                            # one guaranteed pad slot
    nblk1 = (K1pos + P - 1) // P
    npos1 = nblk1 * P

    U = np.unique(np.concatenate([s1, d1]))
    nU = len(U)
    nUt = (nU + P - 1) // P
    uidx = np.zeros((P, nUt), np.int32)
    upad = np.zeros(nUt * P, np.int64)
    upad[:nU] = U
    uidx[:, :] = upad.reshape(nUt, P).T
    uindex = np.zeros(N, np.int64)
    uindex[U] = np.arange(nU)

    # layer-1 edge grid (dst -> grid position via rank over specials)
    rank1 = np.empty(K, np.int64)
    rank1[order1] = np.arange(K)
    d1pos = rank1[np.searchsorted(specials, d1)]
    deg1_sorted = np.zeros(npos1, np.int64)
    deg1_sorted[:K] = deg1[order1]
    L1, sbs1, groups1 = _grid(deg1_sorted, npos1)
    S1 = int(sum(L1))
    pe = np.argsort(d1pos, kind="stable")
    pos_s = d1pos[pe]
    val_s = uindex[s1[pe]].astype(np.int16)
    start_of_pos = np.searchsorted(pos_s, np.arange(npos1))
    k = np.arange(len(pos_s)) - start_of_pos[pos_s]
    slot_base1 = np.concatenate([[0], np.cumsum(L1)])[:-1]
    flat_j = (slot_base1[pos_s // P] + k) * P + (pos_s % P)
    idxflat = np.zeros(S1 * P, np.int16)
    idxflat[flat_j] = val_s
    maskflat = np.zeros(S1 * P, np.float32)
    maskflat[flat_j] = 1.0
    l1_mask = np.ascontiguousarray(maskflat.reshape(S1, P).T)
    l1_eidx = np.concatenate(
        [_wrap16(idxflat[sb["slot0"] * P:(sb["slot0"] + sb["S"]) * P]) for sb in sbs1],
        axis=1,
    )
    dv1 = np.zeros(npos1, np.int16)
    dv1[:K] = uindex[spec_by_pos]
    l1_didx = np.concatenate(
        [_wrap16(dv1[P * sb["b0"]:P * sb["b1"]]) for sb in sbs1], axis=1
    )
    dp1 = np.zeros(npos1, np.float32)
    dp1[:K] = (deg1[order1] > 0)
    l1_degpos = np.ascontiguousarray(dp1.reshape(nblk1, P).T)

    # ---- layer 2 structure (sparse-special grid) ----
    npos = ((NPC + P - 1) // P) * P
    nblk2 = npos // P
    ROW_B2 = K + 1
    REPL_LO = K + 2
    percore = []
    for c in range(NCORES):
        sel = (d2 >= c * NPC) & (d2 < (c + 1) * NPC)
        dl = d2[sel] - c * NPC
        sl = s2[sel]
        deg = np.bincount(dl, minlength=NPC)
        spr_all = rowmap[sl]
        m = spr_all > 0
        spd = dl[m]
        spr = spr_all[m]
        deg_sp = np.bincount(spd, minlength=NPC)
        ndef = deg - deg_sp
        key = 2 * deg_sp + (deg == 0)
        order = np.argsort(-key, kind="stable")
        ncomp = int((key > 0).sum())
        percore.append(dict(deg=deg, deg_sp=deg_sp, ndef=ndef, spd=spd,
                            spr=spr, order=order, ncomp=ncomp))
    ncompb = max(1, max((pc["ncomp"] + P - 1) // P for pc in percore))
    assert ncompb * P <= NPC
    Lb = []
    for b in range(ncompb):
        mx = 0
        for pc in percore:
            mx = max(mx, int(pc["deg_sp"][pc["order"][b * P:(b + 1) * P]].max()))
        Lb.append(mx + 2)                 # + default slot + dst slot
    Lb_arr = np.asarray(Lb)
    S2 = int(sum(Lb))
    slot_base = np.concatenate([[0], np.cumsum(Lb)])[:-1]
    groups2 = []
    b = 0
    while b < ncompb:
        b0 = b
        while b < ncompb and Lb[b] == Lb[b0]:
            b += 1
        groups2.append({"b0": b0, "B": b - b0, "L": Lb[b0],
                        "slot_off": int(slot_base[b0])})

    cores = []
    nposc = ncompb * P
    for c, pc in enumerate(percore):
        deg, deg_sp, ndef = pc["deg"], pc["deg_sp"], pc["ndef"]
        spd, spr, order = pc["spd"], pc["spr"], pc["order"]
        rng = np.random.default_rng(1000 + c)
        idxflat = rng.integers(REPL_LO, VTAB, S2 * P).astype(np.int16)
        maskflat = np.zeros(S2 * P, np.float32)
        wtsflat = np.zeros(S2 * P, np.float32)
        rank = np.empty(NPC, np.int64)
        rank[order] = np.arange(NPC)
        # special-edge slots
        pos = rank[spd]
        pe = np.argsort(pos, kind="stable")
        pos_s = pos[pe]
        val_s = spr[pe].astype(np.int16)
        assert pos_s.size == 0 or pos_s.max() < nposc
        start_of_pos = np.searchsorted(pos_s, np.arange(nposc))
        kk = np.arange(len(pos_s)) - start_of_pos[pos_s]
        flat = (slot_base[pos_s // P] + kk) * P + (pos_s % P)
        idxflat[flat] = val_s
        maskflat[flat] = 1.0
        wtsflat[flat] = 1.0
        # default + dst slots for every computed position
        posn = np.arange(nposc)
        nodes = order[posn]
        blkp = posn // P
        prtp = posn % P
        fd = (slot_base[blkp] + Lb_arr[blkp] - 2) * P + prtp
        ft = (slot_base[blkp] + Lb_arr[blkp] - 1) * P + prtp
        nd = ndef[nodes].astype(np.float32)
        is0 = deg[nodes] == 0
        maskflat[fd] = ((nd > 0) | is0).astype(np.float32)
        wtsflat[fd] = np.where(is0, 1.0, nd)
        idxflat[fd] = np.where(is0, np.int16(ROW_B2), idxflat[fd])
        rm = rowmap[c * NPC + nodes]
        hasrm = rm > 0
        idxflat[ft[hasrm]] = rm[hasrm]
        cores.append({"eidx2": _wrap16(idxflat),
                      "mask2": np.ascontiguousarray(
                          maskflat.reshape(S2, P).T),
                      "wts2": np.ascontiguousarray(
                          wtsflat.reshape(S2, P).T),
                      "order": order})

    meta = {
        "K": K, "nblk1": nblk1, "nU": nU, "nUt": nUt,
        "L1": L1, "sbs1": sbs1, "groups1": groups1, "S1": S1,
        "S2": S2, "groups2": groups2, "ncompb": ncompb,
        "nblk2": nblk2, "npos": npos,
    }
    l1 = {"uidx": uidx, "l1_eidx": l1_eidx, "l1_didx": l1_didx,
          "l1_mask": l1_mask, "l1_degpos": l1_degpos}
    return meta, l1, cores


def _emit_group(nc, gw, Gap, mask_ap, adst_ap, B, L, wts_ap=None,
                degpos_ap=None):
    """Segment softmax + weighted sum for B blocks of equal padded degree L.

    Gap: AP view [128, B*L, 128] of the gathered rows (slot-flat).
    Returns msg tile [128, B, 64]."""
    BL = B * L
    asrc = Gap[:, :, 64:65].rearrange("p s o -> p (s o)")        # [128, BL]
    s_t = gw.tile([P, B, L], F32, tag="s_t")
    nc.vector.tensor_tensor(s_t[:], asrc, adst_ap.to_broadcast((P, B, L)),
                            op=OP.add)
    u_t = gw.tile([P, B, L], F32, tag="u_t")
    nc.vector.scalar_tensor_tensor(u_t[:], s_t[:], NEG_SLOPE, s_t[:],
                                   op0=OP.mult, op1=OP.max)
    e2_t = gw.tile([P, B, L], F32, tag="e2_t")
    nc.vector.scalar_tensor_tensor(e2_t[:], u_t[:], BIG, mask_ap,
                                   op0=OP.add, op1=OP.mult)
    mneg = gw.tile([P, B], F32, tag="mneg")
    nc.vector.tensor_reduce(mneg[:], e2_t[:], axis=mybir.AxisListType.X,
                            op=OP.max, negate=True)
    d_t = gw.tile([P, B, L], F32, tag="d_t")
    nc.vector.tensor_tensor(d_t[:], e2_t[:], mneg[:].to_broadcast((P, B, L)),
                            op=OP.add)
    ex_t = gw.tile([P, B, L], F32, tag="ex_t")
    nc.scalar.activation(ex_t[:], d_t[:], AF.Exp)
    if wts_ap is not None:
        exw_t = gw.tile([P, B, L], F32, tag="exw_t")
        nc.vector.tensor_tensor(exw_t[:], ex_t[:], wts_ap, op=OP.mult)
    else:
        exw_t = ex_t
    ssum = gw.tile([P, B], F32, tag="ssum")
    nc.vector.tensor_reduce(ssum[:], exw_t[:], axis=mybir.AxisListType.X,
                            op=OP.add)
    sp = gw.tile([P, B], F32, tag="sp")
    nc.vector.tensor_scalar_add(sp[:], ssum[:], EPS)
    rs = gw.tile([P, B], F32, tag="rs")
    nc.vector.reciprocal(rs[:], sp[:])
    if degpos_ap is not None:
        rsd = gw.tile([P, B], F32, tag="rsd")
        nc.vector.tensor_tensor(rsd[:], rs[:], degpos_ap, op=OP.mult)
    else:
        rsd = rs
    alpha = gw.tile([P, B, L], F32, tag="alpha")
    nc.vector.tensor_tensor(alpha[:], exw_t[:], rsd[:].to_broadcast((P, B, L)),
                            op=OP.mult)
    wr = gw.tile([P, BL, D], F32, tag="wr")
    nc.vector.tensor_tensor(wr[:], Gap[:, :, 0:D],
                            alpha[:].rearrange("p b l -> p (b l)")
                            .to_broadcast((P, BL, D)), op=OP.mult)
    msg = gw.tile([P, B, D], F32, tag="msg")
    nc.vector.tensor_reduce(msg[:], wr[:].rearrange("p (b l) f -> p b f l", b=B),
                            axis=mybir.AxisListType.X, op=OP.add)
    return msg


def build(meta, repeat=1):
    """Build the SPMD Bass program (common across cores)."""
    K = meta["K"]
    nblk1, nUt = meta["nblk1"], meta["nUt"]
    S1, sbs1, groups1 = meta["S1"], meta["sbs1"], meta["groups1"]
    S2, groups2 = meta["S2"], meta["groups2"]
    ncompb, nblk2 = meta["ncompb"], meta["nblk2"]

    nc = bacc.Bacc("TRN2", target_bir_lowering=False, debug=False,
                   num_devices=NCORES)
    dt = nc.dram_tensor
    x_in = dt("x_in", [N, D], F32, kind="ExternalInput").ap()
    W1_in = dt("W1_in", [D, D], F32, kind="ExternalInput").ap()
    W1T_in = dt("W1T_in", [D, D], F32, kind="ExternalInput").ap()
    W2_in = dt("W2_in", [D, D], F32, kind="ExternalInput").ap()
    W2T_in = dt("W2T_in", [D, D], F32, kind="ExternalInput").ap()
    av1_in = dt("av1_in", [D, 2], F32, kind="ExternalInput").ap()
    av2_in = dt("av2_in", [D, 2], F32, kind="ExternalInput").ap()
    b2row_in = dt("b2row_in", [1, D], F32, kind="ExternalInput").ap()
    b1col_in = dt("b1col_in", [D, 1], F32, kind="ExternalInput").ap()
    uidx_in = dt("uidx_in", [P, nUt], I32, kind="ExternalInput").ap()
    l1_eidx_in = dt("l1_eidx_in", [P, 8 * S1], I16, kind="ExternalInput").ap()
    l1_didx_in = dt("l1_didx_in", [P, 8 * nblk1], I16, kind="ExternalInput").ap()
    l1_mask_in = dt("l1_mask_in", [P, S1], F32, kind="ExternalInput").ap()
    l1_degpos_in = dt("l1_degpos_in", [P, nblk1], F32, kind="ExternalInput").ap()
    eidx2_in = dt("eidx2_in", [P, 8 * S2], I16, kind="ExternalInput").ap()
    mask2_in = dt("mask2_in", [P, S2], F32, kind="ExternalInput").ap()
    wts2_in = dt("wts2_in", [P, S2], F32, kind="ExternalInput").ap()
    out_t = dt("out", [meta["npos"], D], F32, kind="ExternalOutput").ap()

    h1tab = dt("h1tab", [nUt * P, P], F32).ap()
    tab = dt("tab", [VTAB, P], F32).ap()

    with tile.TileContext(nc) as tc, ExitStack() as ctx:
        const = ctx.enter_context(tc.tile_pool(name="const", bufs=1))
        psc_ctx = tc.tile_pool(name="psc", bufs=1, space="PSUM")
        psc = psc_ctx.__enter__()

        ident = const.tile([P, P], F32)
        make_identity(nc, ident[:])

        # ---- weights / augmented matrices ----
        W1s = const.tile([D, D], F32)
        nc.sync.dma_start(W1s[:], W1_in[:])
        W1Ts = const.tile([D, D], F32)
        nc.sync.dma_start(W1Ts[:], W1T_in[:])
        W2s = const.tile([D, D], F32)
        nc.sync.dma_start(W2s[:], W2_in[:])
        W2Ts = const.tile([D, D], F32)
        nc.sync.dma_start(W2Ts[:], W2T_in[:])
        av1s = const.tile([D, 2], F32)
        nc.sync.dma_start(av1s[:], av1_in[:])
        av2s = const.tile([D, 2], F32)
        nc.sync.dma_start(av2s[:], av2_in[:])
        b1cols = const.tile([D, 1], F32)
        nc.sync.dma_start(b1cols[:], b1col_in[:])
        b2rows = const.tile([1, D], F32)
        nc.sync.dma_start(b2rows[:], b2row_in[:])

        wt1_p = psc.tile([D, 2], F32, space="PSUM")
        nc.tensor.matmul(wt1_p[:], W1Ts[:], av1s[:], start=True, stop=True)
        wt2_p = psc.tile([D, 2], F32, space="PSUM")
        nc.tensor.matmul(wt2_p[:], W2Ts[:], av2s[:], start=True, stop=True)
        wt2s = const.tile([D, 2], F32)
        nc.vector.tensor_copy(wt2s[:], wt2_p[:])

        W1aug = const.tile([D, D + 2], F32)
        nc.vector.tensor_copy(W1aug[:, 0:D], W1s[:])
        nc.vector.tensor_copy(W1aug[:, D:D + 2], wt1_p[:])

        # SPEC2 [65, 66] = [[W2 | wt2s | wt2d]; [b1@W2+b2 | b1.wt2s | b1.wt2d]]
        SPEC = const.tile([D + 1, D + 2], F32)
        nc.vector.tensor_copy(SPEC[0:D, 0:D], W2s[:])
        nc.vector.tensor_copy(SPEC[0:D, D:D + 2], wt2s[:])
        b1w2_p = psc.tile([1, D], F32, space="PSUM")
        nc.tensor.matmul(b1w2_p[:], b1cols[:], W2s[:], start=True, stop=True)
        nc.vector.tensor_tensor(SPEC[D:D + 1, 0:D], b1w2_p[:], b2rows[:],
                                op=OP.add)
        b1w_p = psc.tile([1, 2], F32, space="PSUM")
        nc.tensor.matmul(b1w_p[:], b1cols[:], wt2s[:], start=True, stop=True)
        nc.vector.tensor_copy(SPEC[D:D + 1, D:D + 2], b1w_p[:])

        # ---- table constant rows: row0 (default), row K+1 (b2), replicas ----
        row0_s = const.tile([1, P], F32)
        nc.vector.memset(row0_s[:], 0.0)
        nc.vector.tensor_copy(row0_s[:, 0:D + 2], SPEC[D:D + 1, :])
        b2r_s = const.tile([1, P], F32)
        nc.vector.memset(b2r_s[:], 0.0)
        nc.vector.tensor_copy(b2r_s[:, 0:D], b2rows[:])
        ones_s = const.tile([1, P], F32)
        nc.vector.memset(ones_s[:], 1.0)
        repl_p = psc.tile([P, P], F32, space="PSUM")
        nc.tensor.matmul(repl_p[:], ones_s[:], row0_s[:], start=True, stop=True)
        repl_s = const.tile([P, P], F32)
        nc.vector.tensor_copy(repl_s[:], repl_p[:])
        defbig = const.tile([P, DB, D], F32)
        for j in range(DB):
            nc.vector.tensor_copy(defbig[:, j, :], repl_s[:, 0:D])

        nc.sync.dma_start(tab[0:1, :], row0_s[:])
        nc.sync.dma_start(tab[K + 1:K + 2, :], b2r_s[:])
        r0 = K + 2
        while r0 < VTAB:
            cnt = min(P, VTAB - r0)
            nc.sync.dma_start(tab[r0:r0 + cnt, :], repl_s[0:cnt, :])
            r0 += cnt

        psc_ctx.__exit__(None, None, None)

        # ---- index tensors ----
        uidx_s = const.tile([P, nUt], I32)
        nc.sync.dma_start(uidx_s[:], uidx_in[:])
        l1_eidx_s = const.tile([P, 8 * S1], I16)
        nc.sync.dma_start(l1_eidx_s[:], l1_eidx_in[:])
        l1_didx_s = const.tile([P, 8 * nblk1], I16)
        nc.sync.dma_start(l1_didx_s[:], l1_didx_in[:])
        l1_mask_s = const.tile([P, S1], F32)
        nc.sync.dma_start(l1_mask_s[:], l1_mask_in[:])
        l1_degpos_s = const.tile([P, nblk1], F32)
        nc.sync.dma_start(l1_degpos_s[:], l1_degpos_in[:])
        eidx2_s = const.tile([P, 8 * S2], I16)
        nc.sync.dma_start(eidx2_s[:], eidx2_in[:])
        mask2_s = const.tile([P, S2], F32)
        nc.sync.dma_start(mask2_s[:], mask2_in[:])
        wts2_s = const.tile([P, S2], F32)
        nc.sync.dma_start(wts2_s[:], wts2_in[:])

        for _rep in range(repeat):
            # ---- default-region output: constant row broadcast (overlaps) ----
            b = ncompb
            while b < nblk2:
                nb = min(DB, nblk2 - b)
                nc.sync.dma_start(
                    out_t[b * P:(b + nb) * P, :].rearrange(
                        "(k p) f -> p k f", p=P),
                    defbig[:, 0:nb, :])
                b += nb

            # ---- layer 1: h1 table for the U endpoint nodes ----
            with tc.tile_pool(name="l1u", bufs=2) as l1u, \
                 tc.tile_pool(name="l1up", bufs=2, space="PSUM") as l1up:
                for t in range(nUt):
                    xU = l1u.tile([P, D], F32, tag="xU")
                    nc.gpsimd.indirect_dma_start(
                        out=xU[:], out_offset=None, in_=x_in[:, :],
                        in_offset=bass.IndirectOffsetOnAxis(
                            ap=uidx_s[:, t:t + 1], axis=0))
                    xT_p = l1up.tile([D, P], F32, space="PSUM", tag="xT")
                    nc.tensor.transpose(xT_p[:], xU[:], ident[:])
                    xT_s = l1u.tile([D, P], F32, tag="xTs")
                    nc.vector.tensor_copy(xT_s[:], xT_p[:])
                    h_p = l1up.tile([P, D + 2], F32, space="PSUM", tag="h_p")
                    nc.tensor.matmul(h_p[:], xT_s[:], W1aug[:], start=True,
                                     stop=True)
                    h_s = l1u.tile([P, P], F32, tag="h_s")
                    nc.scalar.copy(h_s[:, 0:D + 2], h_p[:])
                    nc.vector.memset(h_s[:, D + 2:P], 0.0)
                    nc.sync.dma_start(h1tab[t * P:(t + 1) * P, :], h_s[:])

            # ---- layer 1 conv -> write special table rows 1..K ----
            with tc.tile_pool(name="l1w", bufs=2) as l1w, \
                 tc.tile_pool(name="l1p", bufs=2, space="PSUM") as l1p:
                dr1 = l1w.tile([P, nblk1, P], F32, tag="dr1")
                nc.gpsimd.dma_gather(dr1[:], h1tab[:, :], l1_didx_s[:],
                                     nblk1 * P, nblk1 * P, P,
                                     single_packet=False)
                adst1 = l1w.tile([P, nblk1], F32, tag="adst1")
                nc.scalar.activation(
                    adst1[:],
                    dr1[:, 0:nblk1, 65:66].rearrange("p b o -> p (b o)"),
                    AF.Identity)
                for sb_i, sb in enumerate(sbs1):
                    G1 = l1w.tile([P, sb["S"], P], F32, tag="G1")
                    nc.gpsimd.dma_gather(
                        G1[:], h1tab[:, :],
                        l1_eidx_s[:, 8 * sb["slot0"]:8 * (sb["slot0"] + sb["S"])],
                        sb["S"] * P, sb["S"] * P, P, single_packet=False)
                    for g in [g for g in groups1 if g["sb"] == sb_i]:
                        B, L, off = g["B"], g["L"], g["slot_off"]
                        sl0 = sb["slot0"] + off
                        msg = _emit_group(
                            nc, l1w, G1[:, off:off + B * L, :],
                            l1_mask_s[:, sl0:sl0 + B * L],
                            adst1[:, g["b0"]:g["b0"] + B],
                            B, L,
                            degpos_ap=l1_degpos_s[:, g["b0"]:g["b0"] + B])
                        for j in range(B):
                            b = g["b0"] + j
                            nrows = min(P, K - b * P)
                            if nrows <= 0:
                                continue
                            mT_p = l1p.tile([D, P], F32, space="PSUM", tag="mT")
                            nc.tensor.transpose(mT_p[:], msg[:, j, :], ident[:])
                            mT_s = l1w.tile([D + 1, P], F32, tag="mTs")
                            nc.vector.tensor_copy(mT_s[0:D, :], mT_p[:])
                            nc.vector.memset(mT_s[D:D + 1, :], 1.0)
                            row_p = l1p.tile([P, D + 2], F32, space="PSUM",
                                             tag="rowp")
                            nc.tensor.matmul(row_p[:], mT_s[:], SPEC[:],
                                             start=True, stop=True)
                            row_s = l1w.tile([P, P], F32, tag="rows")
                            nc.scalar.copy(row_s[:, 0:D + 2], row_p[:])
                            nc.vector.memset(row_s[:, D + 2:P], 0.0)
                            nc.sync.dma_start(
                                tab[1 + b * P:1 + b * P + nrows, :],
                                row_s[0:nrows, :])

            # ---- layer 2: gather slot rows, segment softmax, write out ----
            with tc.tile_pool(name="gw", bufs=2) as gw:
                G = gw.tile([P, S2, P], F32, tag="G")
                nc.gpsimd.dma_gather(G[:], tab[:, :], eidx2_s[:],
                                     S2 * P, S2 * P, P, single_packet=False)
                for g in groups2:
                    B, L, off = g["B"], g["L"], g["slot_off"]
                    Gap = G[:, off:off + B * L, :]
                    adst_g = gw.tile([P, B], F32, tag="adst_g")
                    nc.scalar.activation(
                        adst_g[:],
                        Gap[:, :, 65:66]
                        .rearrange("p (b l) o -> p b (l o)", l=L)[:, :, L - 1:L]
                        .rearrange("p b o -> p (b o)"),
                        AF.Identity)
                    msg = _emit_group(
                        nc, gw, Gap,
                        mask2_s[:, off:off + B * L],
                        adst_g[:], B, L,
                        wts_ap=wts2_s[:, off:off + B * L])
                    nc.sync.dma_start(
                        out_t[g["b0"] * P:(g["b0"] + B) * P, :].rearrange(
                            "(k p) f -> p k f", p=P),
                        msg[:])

    nc.compile()
    return nc


def make_in_maps(inputs, meta, l1, cores):
    x = np.ascontiguousarray(np.asarray(inputs["x"], dtype=np.float32))
    W1 = np.asarray(inputs["W1"], dtype=np.float32)
    W2 = np.asarray(inputs["W2"], dtype=np.float32)
    base = {
        "x_in": x,
        "W1_in": np.ascontiguousarray(W1),
        "W1T_in": np.ascontiguousarray(W1.T),
        "W2_in": np.ascontiguousarray(W2),
        "W2T_in": np.ascontiguousarray(W2.T),
        "av1_in": np.ascontiguousarray(np.stack(
            [np.asarray(inputs["a_src1"]), np.asarray(inputs["a_dst1"])],
            axis=1).astype(np.float32)),
        "av2_in": np.ascontiguousarray(np.stack(
            [np.asarray(inputs["a_src2"]), np.asarray(inputs["a_dst2"])],
            axis=1).astype(np.float32)),
        "b2row_in": np.asarray(inputs["b2"], dtype=np.float32).reshape(1, D),
        "b1col_in": np.asarray(inputs["b1"], dtype=np.float32).reshape(D, 1),
        "uidx_in": l1["uidx"],
        "l1_eidx_in": l1["l1_eidx"],
        "l1_didx_in": l1["l1_didx"],
        "l1_mask_in": l1["l1_mask"],
        "l1_degpos_in": l1["l1_degpos"],
    }
    in_maps = []
    for c in range(NCORES):
        m = dict(base)
        m["eidx2_in"] = cores[c]["eidx2"]
        m["mask2_in"] = cores[c]["mask2"]
        m["wts2_in"] = cores[c]["wts2"]
        in_maps.append(m)
    return in_maps


def unshard(results, cores):
    out = np.empty((N, D), np.float32)
    for c in range(NCORES):
        oc = results[c]["out"]
        order = cores[c]["order"]
        out[c * NPC + order] = oc[:NPC]
    return out


def kernel(**inputs):
    meta, l1, cores = prep(inputs)
    nc = build(meta, repeat=1)
    in_maps = make_in_maps(inputs, meta, l1, cores)
    res = run_bass_kernel_spmd(nc, in_maps, core_ids=list(range(NCORES)))
    return unshard(res.results, cores)


# revision 18
# speedup vs baseline: 10.1816x; 2.0308x over previous
"""GAT 2-layer encoder on 8 Trainium2 NeuronCores.

Reference computation: layer 1 = GAT conv over edge_index[:, :500] (weights W1),
layer 2 = GAT conv over edge_index[:, 500:] (weights W2).

Strategy (sparse-special):
  - Layer-1 output x1 differs from b1 only on the K<=500 distinct dsts of the
    first 500 edges ("specials").  In layer 2, h2[src] = x1[src]@W2 is the
    constant default row for every non-special src, so only edges whose src is
    special (~8k of 1.6M) carry information.  For a dst with no special
    in-edge, softmax over equal scores gives alpha = 1/deg for every in-edge,
    hence out = b1@W2 + b2 exactly (up to the 1e-16 eps), a CONSTANT row.
  - Device builds a (K+2)-row table in h2-space with b2 baked in:
    row r = [x1_r@W2 + b2 | asrc2_r | adst2_r], row 0 = default, row K+1 = b2
    (for deg-0 dsts), rows K+2.. = replicas of row 0 (spread gather load).
  - Sharding: dst-range partition of the 1.6M layer-2 edges across 8 cores (no
    collectives; layer 1 + table build replicated on every core, it is tiny).
  - Per core: dsts sorted so that special-adst / deg-0 dsts land in block 0
    (full slot grid: special slots + default slot + dst slot, all gathered),
    remaining computed blocks carry ONLY special-edge slots; their default
    in-edge mass and adst2 = c_d are handled with per-position scalars.  The
    ~90 all-default blocks are written with one broadcast DMA of the constant
    row.  Layer-2 table rows are pre-transformed by W2, so no matmul there.
"""

import sys

sys.path.insert(0, "/opt/trn_rl_repo")

from contextlib import ExitStack

import numpy as np

import concourse.bacc as bacc
import concourse.bass as bass
import concourse.mybir as mybir
import concourse.tile as tile
from concourse.bass_utils import run_bass_kernel_spmd
from concourse.masks import make_identity

F32 = mybir.dt.float32
I16 = mybir.dt.int16
I32 = mybir.dt.int32
AF = mybir.ActivationFunctionType
OP = mybir.AluOpType

N = 100000
D = 64
NCORES = 8
NPC = N // NCORES          # dst nodes per core
P = 128
NSPLIT = 500               # first 500 edges -> layer 1
NEG_SLOPE = 0.2
BIG = 200.0                # score shift so padded slots underflow exp to 0.0
VTAB = 1024                # gather table rows (specials + default replicas)
NREPL = 512                # default-row replicas written (one broadcast DMA)


def _wrap16(flat):
    """int16 stream [n] (n%16==0) -> dma_gather idx tile [128, n//16]."""
    w = flat.reshape(-1, 16).T
    return np.ascontiguousarray(np.tile(w, (8, 1)).astype(np.int16))


def _groups_of(Ls, b0=0):
    """Split the per-block padded-degree profile into equal-L runs."""
    groups = []
    off = 0
    b = 0
    while b < len(Ls):
        s = b
        while b < len(Ls) and Ls[b] == Ls[s]:
            b += 1
        groups.append({"b0": b0 + s, "B": b - s, "L": Ls[s], "slot_off": off})
        off += (b - s) * Ls[s]
    return groups


def prep(inputs):
    """Host-side index prep (pure index computation, no feature values)."""
    ei = np.asarray(inputs["edge_index"])
    src = ei[0].astype(np.int64)
    dst = ei[1].astype(np.int64)
    s1, d1 = src[:NSPLIT], dst[:NSPLIT]
    s2, d2 = src[NSPLIT:], dst[NSPLIT:]

    # ---- layer 1 structure ----
    specials, deg1 = np.unique(d1, return_counts=True)
    K = len(specials)
    order1 = np.argsort(-deg1, kind="stable")
    spec_by_pos = specials[order1]          # grid position q -> node, table row q+1
    rowmap = np.zeros(N, np.int16)
    rowmap[spec_by_pos] = np.arange(1, K + 1)
    nblk1 = (K + P - 1) // P
    npos1 = nblk1 * P

    U = np.unique(np.concatenate([s1, d1]))
    nU = len(U)
    # x-row gather in int16-addressable ranges of 32768 rows
    RSPAN = 1 << 15
    xranges = []          # (lo, ntiles)
    uidx16_parts = []
    uindex = np.zeros(N, np.int64)
    off = 0
    for lo in range(0, N, RSPAN):
        hi = min(lo + RSPAN, N)
        Ur = U[(U >= lo) & (U < hi)]
        if len(Ur) == 0:
            continue
        nt = (len(Ur) + P - 1) // P
        pad = np.full(nt * P, lo, np.int64)
        pad[:len(Ur)] = Ur
        uindex[Ur] = off * P + np.arange(len(Ur))
        uidx16_parts.append(_wrap16((pad - lo).astype(np.int16)))
        xranges.append((lo, nt))
        off += nt
    nUt = off
    uidx16 = np.concatenate(uidx16_parts, axis=1)

    # layer-1 slot grid: per block [special-edge slots | dst slot]
    rank1 = np.empty(K, np.int64)
    rank1[order1] = np.arange(K)
    d1pos = rank1[np.searchsorted(specials, d1)]
    deg1_sorted = np.zeros(npos1, np.int64)
    deg1_sorted[:K] = deg1[order1]
    L1sp = [max(int(deg1_sorted[b * P:(b + 1) * P].max()), 1)
            for b in range(nblk1)]
    L1 = [l + 1 for l in L1sp]
    S1 = int(sum(L1))
    slot_base1 = np.concatenate([[0], np.cumsum(L1)])[:-1]
    idx1 = np.zeros(S1 * P, np.int16)
    mask1 = np.zeros(S1 * P, np.float32)
    pe = np.argsort(d1pos, kind="stable")
    pos_s = d1pos[pe]
    val_s = uindex[s1[pe]].astype(np.int16)
    start_of_pos = np.searchsorted(pos_s, np.arange(npos1))
    kk = np.arange(len(pos_s)) - start_of_pos[pos_s]
    flat = (slot_base1[pos_s // P] + kk) * P + (pos_s % P)
    idx1[flat] = val_s
    mask1[flat] = 1.0
    # dst slots (last slot of each block)
    posn = np.arange(npos1)
    dv1 = np.zeros(npos1, np.int16)
    dv1[:K] = uindex[spec_by_pos]
    fdst = (slot_base1[posn // P] + np.asarray(L1)[posn // P] - 1) * P + posn % P
    idx1[fdst] = dv1
    dp1 = np.zeros(npos1, np.float32)
    dp1[:K] = 1.0
    l1_eidx = _wrap16(idx1)
    l1_f = np.concatenate(
        [np.ascontiguousarray(mask1.reshape(S1, P).T),
         np.ascontiguousarray(dp1.reshape(nblk1, P).T)], axis=1)
    groups1 = _groups_of(L1)

    # ---- layer 2 structure (sparse-special grid) ----
    npos = ((NPC + P - 1) // P) * P
    nblk2 = npos // P
    ROW_B2 = K + 1
    REPL_LO = K + 2
    REPL_HI = REPL_LO + NREPL
    assert REPL_HI <= VTAB
    percore = []
    for c in range(NCORES):
        sel = (d2 >= c * NPC) & (d2 < (c + 1) * NPC)
        dl = d2[sel] - c * NPC
        sl = s2[sel]
        deg = np.bincount(dl, minlength=NPC)
        spr_all = rowmap[sl]
        m = spr_all > 0
        spd = dl[m]
        spr = spr_all[m]
        deg_sp = np.bincount(spd, minlength=NPC)
        ndef = deg - deg_sp
        spadst = rowmap[c * NPC:(c + 1) * NPC] > 0
        front = spadst | (deg == 0)
        assert int(front.sum()) <= P
        key = front.astype(np.int64) * (1 << 20) + deg_sp
        order = np.argsort(-key, kind="stable")
        ncomp = int((key > 0).sum())
        percore.append(dict(deg=deg, deg_sp=deg_sp, ndef=ndef, spd=spd,
                            spr=spr, order=order, ncomp=ncomp))
    ncompb = max(1, max((pc["ncomp"] + P - 1) // P for pc in percore))
    assert ncompb * P <= NPC

    def blkmax(pc, b):
        return int(pc["deg_sp"][pc["order"][b * P:(b + 1) * P]].max())

    L0 = max(blkmax(pc, 0) for pc in percore) + 2   # +default +dst slot
    Lb = [max(max(blkmax(pc, b) for pc in percore), 1)
          for b in range(1, ncompb)]
    S_g = L0 + int(sum(Lb))
    slot_base = np.zeros(ncompb, np.int64)                 # per-block slot base
    if ncompb > 1:
        slot_base[1:] = L0 + np.concatenate([[0], np.cumsum(Lb)[:-1]])
    cap = np.asarray([L0 - 2] + Lb)                        # special capacity
    groups2b = _groups_of(Lb, b0=1)

    cores = []
    nposc = ncompb * P
    for c, pc in enumerate(percore):
        deg, deg_sp, ndef = pc["deg"], pc["deg_sp"], pc["ndef"]
        spd, spr, order = pc["spd"], pc["spr"], pc["order"]
        rng = np.random.default_rng(1000 + c)
        idxflat = rng.integers(REPL_LO, REPL_HI, S_g * P).astype(np.int16)
        maskflat = np.zeros(S_g * P, np.float32)
        wts0 = np.zeros(L0 * P, np.float32)
        rank = np.empty(NPC, np.int64)
        rank[order] = np.arange(NPC)
        # special-edge slots
        pos = rank[spd]
        pe = np.argsort(pos, kind="stable")
        pos_s = pos[pe]
        val_s = spr[pe].astype(np.int16)
        assert pos_s.size == 0 or pos_s.max() < nposc
        start_of_pos = np.searchsorted(pos_s, np.arange(nposc))
        kk = np.arange(len(pos_s)) - start_of_pos[pos_s]
        assert np.all(kk < cap[pos_s // P])
        flat = (slot_base[pos_s // P] + kk) * P + (pos_s % P)
        idxflat[flat] = val_s
        maskflat[flat] = 1.0
        # block 0: default + dst slots
        p0 = np.arange(P)
        nodes0 = order[p0]
        nd0 = ndef[nodes0].astype(np.float32)
        is00 = deg[nodes0] == 0
        fd = (L0 - 2) * P + p0
        ft = (L0 - 1) * P + p0
        maskflat[fd] = ((nd0 > 0) | is00).astype(np.float32)
        wts0[0:(L0 - 2) * P] = maskflat[0:(L0 - 2) * P]
        wts0[fd] = np.where(is00, 1.0, nd0)
        idxflat[fd] = np.where(is00, np.int16(ROW_B2), idxflat[fd])
        rm0 = rowmap[c * NPC + nodes0]
        h0 = rm0 > 0
        idxflat[ft[h0]] = rm0[h0]
        # blocks >= 1 scalar fields
        posn = np.arange(P, nposc)
        nodesb = order[posn]
        assert np.all(deg[nodesb] > 0)
        m0b = (ndef[nodesb] > 0).astype(np.float32)
        w0b = ndef[nodesb].astype(np.float32)
        m0b_t = np.ascontiguousarray(m0b.reshape(ncompb - 1, P).T) \
            if ncompb > 1 else np.zeros((P, 0), np.float32)
        w0b_t = np.ascontiguousarray(w0b.reshape(ncompb - 1, P).T) \
            if ncompb > 1 else np.zeros((P, 0), np.float32)
        l2f = np.concatenate(
            [np.ascontiguousarray(maskflat.reshape(S_g, P).T),
             np.ascontiguousarray(wts0.reshape(L0, P).T),
             m0b_t, w0b_t], axis=1)
        cores.append({"eidx2": _wrap16(idxflat), "l2f": l2f, "order": order})

    meta = {
        "K": K, "nblk1": nblk1, "nU": nU, "nUt": nUt, "xranges": xranges,
        "L1": L1, "groups1": groups1, "S1": S1,
        "L0": L0, "Lb": Lb, "S_g": S_g, "groups2b": groups2b,
        "ncompb": ncompb, "nblk2": nblk2, "npos": npos,
        "ROW_B2": ROW_B2, "REPL_LO": REPL_LO,
    }
    l1 = {"uidx16": uidx16, "l1_eidx": l1_eidx, "l1_f": l1_f}
    return meta, l1, cores


def _extract_lastslot(nc, gw, Gap, B, L, col, tag):
    """[P, B] tile holding Gap[:, b*L + L-1, col] per block b."""
    t = gw.tile([P, B], F32, tag=tag)
    nc.scalar.activation(
        t[:],
        Gap[:, :, col:col + 1]
        .rearrange("p (b l) o -> p b (l o)", l=L)[:, :, L - 1:L]
        .rearrange("p b o -> p (b o)"),
        AF.Identity)
    return t


def _emit_group(nc, gw, Gap, mask_ap, adst_ap, B, L, wts_ap=None,
                degpos_ap=None):
    """Segment softmax + weighted sum for B blocks of equal padded degree L.

    Gap: AP view [128, B*L, 128] of the gathered rows (slot-flat).
    Returns msg tile [128, B, 64]."""
    BL = B * L
    asrc = Gap[:, :, 64:65].rearrange("p s o -> p (s o)")        # [128, BL]
    s_t = gw.tile([P, B, L], F32, tag="s_t")
    nc.vector.tensor_tensor(s_t[:], asrc, adst_ap.to_broadcast((P, B, L)),
                            op=OP.add)
    u_t = gw.tile([P, B, L], F32, tag="u_t")
    nc.vector.scalar_tensor_tensor(u_t[:], s_t[:], NEG_SLOPE, s_t[:],
                                   op0=OP.mult, op1=OP.max)
    e2_t = gw.tile([P, B, L], F32, tag="e2_t")
    nc.vector.scalar_tensor_tensor(e2_t[:], u_t[:], BIG, mask_ap,
                                   op0=OP.add, op1=OP.mult)
    mneg = gw.tile([P, B], F32, tag="mneg")
    nc.vector.tensor_reduce(mneg[:], e2_t[:], axis=mybir.AxisListType.X,
                            op=OP.max, negate=True)
    d_t = gw.tile([P, B, L], F32, tag="d_t")
    nc.vector.tensor_tensor(d_t[:], e2_t[:], mneg[:].to_broadcast((P, B, L)),
                            op=OP.add)
    ex_t = gw.tile([P, B, L], F32, tag="ex_t")
    nc.scalar.activation(ex_t[:], d_t[:], AF.Exp)
    if wts_ap is not None:
        exw_t = gw.tile([P, B, L], F32, tag="exw_t")
        nc.vector.tensor_tensor(exw_t[:], ex_t[:], wts_ap, op=OP.mult)
    else:
        exw_t = ex_t
    ssum = gw.tile([P, B], F32, tag="ssum")
    nc.vector.tensor_reduce(ssum[:], exw_t[:], axis=mybir.AxisListType.X,
                            op=OP.add)
    rs = gw.tile([P, B], F32, tag="rs")
    nc.vector.reciprocal(rs[:], ssum[:])
    if degpos_ap is not None:
        rsd = gw.tile([P, B], F32, tag="rsd")
        nc.vector.tensor_tensor(rsd[:], rs[:], degpos_ap, op=OP.mult)
    else:
        rsd = rs
    alpha = gw.tile([P, B, L], F32, tag="alpha")
    nc.vector.tensor_tensor(alpha[:], exw_t[:], rsd[:].to_broadcast((P, B, L)),
                            op=OP.mult)
    wr = gw.tile([P, BL, D], F32, tag="wr")
    nc.vector.tensor_tensor(wr[:], Gap[:, :, 0:D],
                            alpha[:].rearrange("p b l -> p (b l)")
                            .to_broadcast((P, BL, D)), op=OP.mult)
    msg = gw.tile([P, B, D], F32, tag="msg")
    nc.vector.tensor_reduce(msg[:], wr[:].rearrange("p (b l) f -> p b f l", b=B),
                            axis=mybir.AxisListType.X, op=OP.add)
    return msg


def _emit_sp_group(nc, gw, Gap, mask_ap, m0b_ap, w0b_ap, e0c, cdrep, defrowv,
                   B, L):
    """Blocks with only special-edge slots: default-edge mass via scalars.

    Gap [128, B*L, 128]; adst = c_d (non-special dsts); e0c [P,1] = default
    score lrelu(c_s+c_d)+BIG; m0b/w0b [P,B] = (ndef>0) and ndef.
    Returns msg tile [128, B, 64] (includes the default-row contribution)."""
    BL = B * L
    asrc = Gap[:, :, 64:65].rearrange("p s o -> p (s o)")
    s_t = gw.tile([P, B, L], F32, tag="sp_s")
    nc.vector.tensor_tensor(s_t[:], asrc, cdrep.to_broadcast((P, B, L)),
                            op=OP.add)
    u_t = gw.tile([P, B, L], F32, tag="sp_u")
    nc.vector.scalar_tensor_tensor(u_t[:], s_t[:], NEG_SLOPE, s_t[:],
                                   op0=OP.mult, op1=OP.max)
    e2_t = gw.tile([P, B, L], F32, tag="sp_e2")
    nc.vector.scalar_tensor_tensor(e2_t[:], u_t[:], BIG, mask_ap,
                                   op0=OP.add, op1=OP.mult)
    e0e = gw.tile([P, B], F32, tag="sp_e0")
    nc.vector.tensor_tensor(e0e[:], m0b_ap, e0c.to_broadcast((P, B)),
                            op=OP.mult)
    m_t = gw.tile([P, B], F32, tag="sp_m")
    if L > 1:
        msp = gw.tile([P, B], F32, tag="sp_msp")
        nc.vector.tensor_reduce(msp[:], e2_t[:], axis=mybir.AxisListType.X,
                                op=OP.max)
        nc.vector.tensor_tensor(m_t[:], msp[:], e0e[:], op=OP.max)
    else:
        nc.vector.tensor_tensor(
            m_t[:], e2_t[:].rearrange("p b l -> p (b l)"), e0e[:], op=OP.max)
    d_t = gw.tile([P, B, L], F32, tag="sp_d")
    nc.vector.tensor_tensor(d_t[:], e2_t[:], m_t[:].to_broadcast((P, B, L)),
                            op=OP.subtract)
    ex_t = gw.tile([P, B, L], F32, tag="sp_ex")
    nc.scalar.activation(ex_t[:], d_t[:], AF.Exp)
    d0 = gw.tile([P, B], F32, tag="sp_d0")
    nc.vector.tensor_tensor(d0[:], e0e[:], m_t[:], op=OP.subtract)
    ex0 = gw.tile([P, B], F32, tag="sp_ex0")
    nc.scalar.activation(ex0[:], d0[:], AF.Exp)
    exw0 = gw.tile([P, B], F32, tag="sp_exw0")
    nc.vector.tensor_tensor(exw0[:], ex0[:], w0b_ap, op=OP.mult)
    ssum = gw.tile([P, B], F32, tag="sp_ssum")
    if L > 1:
        ssp = gw.tile([P, B], F32, tag="sp_ssp")
        nc.vector.tensor_reduce(ssp[:], ex_t[:], axis=mybir.AxisListType.X,
                                op=OP.add)
        nc.vector.tensor_tensor(ssum[:], ssp[:], exw0[:], op=OP.add)
    else:
        nc.vector.tensor_tensor(
            ssum[:], ex_t[:].rearrange("p b l -> p (b l)"), exw0[:], op=OP.add)
    rs = gw.tile([P, B], F32, tag="sp_rs")
    nc.vector.reciprocal(rs[:], ssum[:])
    alpha = gw.tile([P, B, L], F32, tag="sp_al")
    nc.vector.tensor_tensor(alpha[:], ex_t[:], rs[:].to_broadcast((P, B, L)),
                            op=OP.mult)
    alpha0 = gw.tile([P, B], F32, tag="sp_al0")
    nc.vector.tensor_tensor(alpha0[:], exw0[:], rs[:], op=OP.mult)
    wr = gw.tile([P, BL, D], F32, tag="sp_wr")
    nc.vector.tensor_tensor(wr[:], Gap[:, :, 0:D],
                            alpha[:].rearrange("p b l -> p (b l)")
                            .to_broadcast((P, BL, D)), op=OP.mult)
    if L > 1:
        msgs = gw.tile([P, B, D], F32, tag="sp_msgs")
        nc.vector.tensor_reduce(
            msgs[:], wr[:].rearrange("p (b l) f -> p b f l", b=B),
            axis=mybir.AxisListType.X, op=OP.add)
    else:
        msgs = wr
    t1 = gw.tile([P, B, D], F32, tag="sp_t1")
    nc.vector.tensor_tensor(t1[:], alpha0[:].to_broadcast((P, B, D)),
                            defrowv.to_broadcast((P, B, D)), op=OP.mult)
    msg = gw.tile([P, B, D], F32, tag="sp_msg")
    nc.vector.tensor_tensor(msg[:], msgs[:], t1[:], op=OP.add)
    return msg


def build(meta, repeat=1, stages="ducge"):
    """Build the SPMD Bass program (common across cores).

    stages: subset of 'd' (default writes), 'u' (l1 endpoint table),
    'c' (l1 conv -> tab rows), 'g' (l2 gather), 'e' (l2 emit+write)."""
    K = meta["K"]
    nblk1, nUt = meta["nblk1"], meta["nUt"]
    S1, groups1, L1 = meta["S1"], meta["groups1"], meta["L1"]
    S_g, L0, groups2b = meta["S_g"], meta["L0"], meta["groups2b"]
    ncompb, nblk2 = meta["ncompb"], meta["nblk2"]
    ROW_B2, REPL_LO = meta["ROW_B2"], meta["REPL_LO"]
    NPRM = 4 * D + 5
    nf1 = S1 + nblk1
    nf2 = S_g + L0 + 2 * (ncompb - 1)

    nc = bacc.Bacc("TRN2", target_bir_lowering=False, debug=False,
                   num_devices=NCORES)
    dt = nc.dram_tensor
    xranges = meta["xranges"]
    x_in = dt("x_in", [N, D], F32, kind="ExternalInput").ap()
    params_in = dt("params_in", [D, NPRM], F32, kind="ExternalInput").ap()
    b2row_in = dt("b2row_in", [1, D], F32, kind="ExternalInput").ap()
    uidx_in = dt("uidx_in", [P, 8 * nUt], I16, kind="ExternalInput").ap()
    l1_eidx_in = dt("l1_eidx_in", [P, 8 * S1], I16, kind="ExternalInput").ap()
    l1_f_in = dt("l1_f_in", [P, nf1], F32, kind="ExternalInput").ap()
    eidx2_in = dt("eidx2_in", [P, 8 * S_g], I16, kind="ExternalInput").ap()
    l2f_in = dt("l2f_in", [P, nf2], F32, kind="ExternalInput").ap()
    out_t = dt("out", [meta["npos"], D], F32, kind="ExternalOutput").ap()

    h1tab = dt("h1tab", [nUt * P, P], F32).ap()
    tab = dt("tab", [VTAB, P], F32).ap()

    with tile.TileContext(nc) as tc, ExitStack() as ctx:
        const = ctx.enter_context(tc.tile_pool(name="const", bufs=1))
        psc_ctx = tc.tile_pool(name="psc", bufs=1, space="PSUM")
        psc = psc_ctx.__enter__()

        ident = const.tile([P, P], F32)
        make_identity(nc, ident[:])

        # ---- parameters (one blob: W1|W1T|W2|W2T|av1|av2|b1col) ----
        prm = const.tile([D, NPRM], F32)
        nc.sync.dma_start(prm[:], params_in[:])
        W1s = prm[:, 0:D]
        W1Ts = prm[:, D:2 * D]
        W2s = prm[:, 2 * D:3 * D]
        W2Ts = prm[:, 3 * D:4 * D]
        av1s = prm[:, 4 * D:4 * D + 2]
        av2s = prm[:, 4 * D + 2:4 * D + 4]
        b1cols = prm[:, 4 * D + 4:4 * D + 5]
        b2rows = const.tile([1, D], F32)
        nc.sync.dma_start(b2rows[:], b2row_in[:])

        wt1_p = psc.tile([D, 2], F32, space="PSUM")
        nc.tensor.matmul(wt1_p[:], W1Ts, av1s, start=True, stop=True)
        wt2_p = psc.tile([D, 2], F32, space="PSUM")
        nc.tensor.matmul(wt2_p[:], W2Ts, av2s, start=True, stop=True)
        wt2s = const.tile([D, 2], F32)
        nc.vector.tensor_copy(wt2s[:], wt2_p[:])

        W1aug = const.tile([D, D + 2], F32)
        nc.vector.tensor_copy(W1aug[:, 0:D], W1s)
        nc.vector.tensor_copy(W1aug[:, D:D + 2], wt1_p[:])

        # SPEC2 [65, 66] = [[W2 | wt2s wt2d]; [b1@W2+b2 | b1.wt2s b1.wt2d]]
        SPEC = const.tile([D + 1, D + 2], F32)
        nc.vector.tensor_copy(SPEC[0:D, 0:D], W2s)
        nc.vector.tensor_copy(SPEC[0:D, D:D + 2], wt2s[:])
        b1w2_p = psc.tile([1, D], F32, space="PSUM")
        nc.tensor.matmul(b1w2_p[:], b1cols, W2s, start=True, stop=True)
        nc.vector.tensor_tensor(SPEC[D:D + 1, 0:D], b1w2_p[:], b2rows[:],
                                op=OP.add)
        b1w_p = psc.tile([1, 2], F32, space="PSUM")
        nc.tensor.matmul(b1w_p[:], b1cols, wt2s[:], start=True, stop=True)
        nc.vector.tensor_copy(SPEC[D:D + 1, D:D + 2], b1w_p[:])

        # ---- constant table rows + replicated default row ----
        row0_s = const.tile([1, P], F32)
        nc.vector.memset(row0_s[:], 0.0)
        nc.vector.tensor_copy(row0_s[:, 0:D + 2], SPEC[D:D + 1, :])
        b2r_s = const.tile([1, P], F32)
        nc.vector.memset(b2r_s[:], 0.0)
        nc.vector.tensor_copy(b2r_s[:, 0:D], b2rows[:])
        ones_s = const.tile([1, P], F32)
        nc.vector.memset(ones_s[:], 1.0)
        repl_p = psc.tile([P, P], F32, space="PSUM")
        nc.tensor.matmul(repl_p[:], ones_s[:], row0_s[:], start=True, stop=True)
        repl_s = const.tile([P, P], F32)
        nc.vector.tensor_copy(repl_s[:], repl_p[:])
        defrowv = repl_s[:, 0:D].rearrange("p (k f) -> p k f", k=1)
        csrep = repl_s[:, D:D + 1]
        cdrep = repl_s[:, D + 1:D + 2]
        replv = repl_s[:].rearrange("p (k f) -> p k f", k=1)
        KB = 16                      # blocks per default-write chunk
        defbig = const.tile([P, KB * D], F32)
        nc.vector.tensor_copy(
            defbig[:].rearrange("p (k f) -> p k f", k=KB),
            defrowv.to_broadcast((P, KB, D)))

        nc.sync.dma_start(tab[0:1, :], row0_s[:])
        nc.sync.dma_start(tab[ROW_B2:ROW_B2 + 1, :], b2r_s[:])
        nc.sync.dma_start(
            tab[REPL_LO:REPL_LO + NREPL, :].rearrange("(k p) f -> p k f", p=P),
            replv.to_broadcast((P, NREPL // P, P)))

        # default-score constant e0c = lrelu(c_s + c_d) + BIG
        s0c = const.tile([P, 1], F32)
        nc.vector.tensor_tensor(s0c[:], csrep, cdrep, op=OP.add)
        u0c = const.tile([P, 1], F32)
        nc.vector.scalar_tensor_tensor(u0c[:], s0c[:], NEG_SLOPE, s0c[:],
                                       op0=OP.mult, op1=OP.max)
        e0c = const.tile([P, 1], F32)
        nc.vector.tensor_scalar_add(e0c[:], u0c[:], BIG)

        psc_ctx.__exit__(None, None, None)

        # ---- index tensors ----
        uidx_s = const.tile([P, 8 * nUt], I16)
        nc.sync.dma_start(uidx_s[:], uidx_in[:])
        l1_eidx_s = const.tile([P, 8 * S1], I16)
        nc.sync.dma_start(l1_eidx_s[:], l1_eidx_in[:])
        l1_f_s = const.tile([P, nf1], F32)
        nc.sync.dma_start(l1_f_s[:], l1_f_in[:])
        l1_mask_s = l1_f_s[:, 0:S1]
        l1_degpos_s = l1_f_s[:, S1:S1 + nblk1]
        eidx2_s = const.tile([P, 8 * S_g], I16)
        nc.sync.dma_start(eidx2_s[:], eidx2_in[:])
        l2f_s = const.tile([P, nf2], F32)
        nc.sync.dma_start(l2f_s[:], l2f_in[:])
        mask2_s = l2f_s[:, 0:S_g]
        wts0_s = l2f_s[:, S_g:S_g + L0]
        m0b_s = l2f_s[:, S_g + L0:S_g + L0 + (ncompb - 1)]
        w0b_s = l2f_s[:, S_g + L0 + (ncompb - 1):nf2]

        # persistent staging tiles (values rewritten every rep)
        h_all = const.tile([P, nUt, D + 2], F32)
        mTs_c = const.tile([D + 1, nblk1, P], F32)
        nc.vector.memset(mTs_c[D:D + 1, :, :], 1.0)
        row_all = const.tile([P, nblk1, D + 2], F32)

        ndefblk = nblk2 - ncompb
        outcmp = out_t[0:ncompb * P, :].rearrange("(p b) f -> p b f", b=ncompb)

        for _rep in range(repeat):
            # ---- default-region output: chunked contiguous writes ----
            if "d" in stages:
                b = 0
                while b < ndefblk:
                    nb = min(KB, ndefblk - b)
                    r0 = (ncompb + b) * P
                    nc.sync.dma_start(
                        out_t[r0:r0 + nb * P, :].rearrange(
                            "(p k) f -> p (k f)", k=nb),
                        defbig[:, 0:nb * D])
                    b += nb

            # ---- layer 1: h1 table for the U endpoint nodes ----
            if "u" not in stages:
                continue
            with tc.tile_pool(name="l1u", bufs=2) as l1u, \
                 tc.tile_pool(name="l1up", bufs=4, space="PSUM") as l1up:
                xall = l1u.tile([P, nUt, D], F32, tag="xall")
                toff = 0
                for lo, nt in xranges:
                    hi = min(lo + (1 << 15), N)
                    nc.gpsimd.dma_gather(
                        xall[:, toff:toff + nt, :], x_in[lo:hi, :],
                        uidx_s[:, 8 * toff:8 * (toff + nt)],
                        nt * P, nt * P, D, single_packet=False)
                    toff += nt
                for t in range(nUt):
                    xT_p = l1up.tile([D, P], F32, space="PSUM", tag="xT")
                    nc.tensor.transpose(xT_p[:], xall[:, t, :], ident[:])
                    xT_s = l1u.tile([D, P], F32, tag="xTs")
                    nc.vector.tensor_copy(xT_s[:], xT_p[:])
                    h_p = l1up.tile([P, D + 2], F32, space="PSUM", tag="h_p")
                    nc.tensor.matmul(h_p[:], xT_s[:], W1aug[:], start=True,
                                     stop=True)
                    nc.scalar.copy(h_all[:, t, :], h_p[:])
                nc.sync.dma_start(
                    h1tab[:, 0:D + 2].rearrange("(k p) f -> p k f", p=P),
                    h_all[:])

            # ---- layer 1 conv -> write special table rows 1..K ----
            if "c" not in stages:
                continue
            with tc.tile_pool(name="l1w", bufs=2) as l1w, \
                 tc.tile_pool(name="l1p", bufs=4, space="PSUM") as l1p:
                G1 = l1w.tile([P, S1, P], F32, tag="G1")
                nc.gpsimd.dma_gather(G1[:], h1tab[:, :], l1_eidx_s[:],
                                     S1 * P, S1 * P, P, single_packet=False)
                for g in groups1:
                    B, L, off = g["B"], g["L"], g["slot_off"]
                    Gap = G1[:, off:off + B * L, :]
                    adst1 = _extract_lastslot(nc, l1w, Gap, B, L, D + 1,
                                              "adst1")
                    msg = _emit_group(
                        nc, l1w, Gap, l1_mask_s[:, off:off + B * L],
                        adst1[:], B, L,
                        degpos_ap=l1_degpos_s[:, g["b0"]:g["b0"] + B])
                    for j in range(B):
                        b = g["b0"] + j
                        mT_p = l1p.tile([D, P], F32, space="PSUM", tag="mT")
                        nc.tensor.transpose(mT_p[:], msg[:, j, :], ident[:])
                        nc.vector.tensor_copy(mTs_c[0:D, b, :], mT_p[:])
                        row_p = l1p.tile([P, D + 2], F32, space="PSUM",
                                         tag="rowp")
                        nc.tensor.matmul(row_p[:], mTs_c[:, b, :], SPEC[:],
                                         start=True, stop=True)
                        nc.scalar.copy(row_all[:, b, :], row_p[:])
                nfull = K // P
                if nfull:
                    nc.sync.dma_start(
                        tab[1:1 + nfull * P, 0:D + 2].rearrange(
                            "(k p) f -> p k f", p=P),
                        row_all[:, 0:nfull, :])
                rem = K - nfull * P
                if rem:
                    nc.sync.dma_start(tab[1 + nfull * P:1 + K, 0:D + 2],
                                      row_all[0:rem, nfull, :])

            # ---- layer 2 ----
            if "g" not in stages:
                continue
            with tc.tile_pool(name="gw", bufs=2) as gw:
                G = gw.tile([P, S_g, P], F32, tag="G")
                nc.gpsimd.dma_gather(G[:], tab[:, :], eidx2_s[:],
                                     S_g * P, S_g * P, P, single_packet=False)
                if "e" not in stages:
                    dum = gw.tile([P, P], F32, tag="dum")
                    nc.vector.tensor_copy(dum[:], G[:, 0, :])
                    continue
                # block 0: full grid
                Gap0 = G[:, 0:L0, :]
                adst0 = _extract_lastslot(nc, gw, Gap0, 1, L0, D + 1, "adst0")
                msg0 = _emit_group(nc, gw, Gap0, mask2_s[:, 0:L0], adst0[:],
                                   1, L0, wts_ap=wts0_s[:])
                nc.sync.dma_start(outcmp[:, 0:1, :], msg0[:])
                # blocks >= 1: special-only grids
                for g in groups2b:
                    B, L, off = g["B"], g["L"], g["slot_off"]
                    Gap = G[:, L0 + off:L0 + off + B * L, :]
                    msg = _emit_sp_group(
                        nc, gw, Gap, mask2_s[:, L0 + off:L0 + off + B * L],
                        m0b_s[:, g["b0"] - 1:g["b0"] - 1 + B],
                        w0b_s[:, g["b0"] - 1:g["b0"] - 1 + B],
                        e0c[:], cdrep, defrowv, B, L)
                    nc.sync.dma_start(outcmp[:, g["b0"]:g["b0"] + B, :],
                                      msg[:])

    nc.compile()
    return nc


def make_in_maps(inputs, meta, l1, cores):
    x = np.ascontiguousarray(np.asarray(inputs["x"], dtype=np.float32))
    W1 = np.asarray(inputs["W1"], dtype=np.float32)
    W2 = np.asarray(inputs["W2"], dtype=np.float32)
    params = np.concatenate(
        [W1, np.ascontiguousarray(W1.T), W2, np.ascontiguousarray(W2.T),
         np.stack([np.asarray(inputs["a_src1"]),
                   np.asarray(inputs["a_dst1"])], axis=1),
         np.stack([np.asarray(inputs["a_src2"]),
                   np.asarray(inputs["a_dst2"])], axis=1),
         np.asarray(inputs["b1"]).reshape(D, 1)],
        axis=1).astype(np.float32)
    base = {
        "x_in": x,
        "params_in": np.ascontiguousarray(params),
        "b2row_in": np.asarray(inputs["b2"], dtype=np.float32).reshape(1, D),
        "uidx_in": l1["uidx16"],
        "l1_eidx_in": l1["l1_eidx"],
        "l1_f_in": l1["l1_f"],
    }
    in_maps = []
    for c in range(NCORES):
        m = dict(base)
        m["eidx2_in"] = cores[c]["eidx2"]
        m["l2f_in"] = cores[c]["l2f"]
        in_maps.append(m)
    return in_maps


def unshard_core(oc, order, ncompb):
    got = np.empty((NPC, D), np.float32)
    nposc = ncompb * P
    pos = np.arange(nposc)
    got[order[:nposc]] = oc[(pos % P) * ncompb + pos // P]
    got[order[nposc:NPC]] = oc[nposc:NPC]
    return got


def unshard(results, cores, meta):
    out = np.empty((N, D), np.float32)
    for c in range(NCORES):
        out[c * NPC:(c + 1) * NPC] = unshard_core(
            results[c]["out"], cores[c]["order"], meta["ncompb"])
    return out


def kernel(**inputs):
    meta, l1, cores = prep(inputs)
    nc = build(meta, repeat=1)
    in_maps = make_in_maps(inputs, meta, l1, cores)
    res = run_bass_kernel_spmd(nc, in_maps, core_ids=list(range(NCORES)))
    return unshard(res.results, cores, meta)
